# revision 1
# baseline (speedup 1.0000x reference)
"""Multi-head attention (RoPE, causal) Trainium2 kernel, 8-way sharded.

Sharding: core c => batch b = c//2, head-group g = c%2 (8 of 16 heads).
Each core computes Q/K/V projections for its (b, g), RoPE, causal
attention over its 8 heads, and the row-slice of the output projection.
Host sums the two partial output projections per batch and adds b_o.

Per-core dataflow (fp32 storage; matmuls in float32r — single-pass
reduced-precision fp32, ~1.7e-4 per-matmul rel err, 2.7x the throughput
of full fp32's two half-speed passes):
  - projections contract over model dim via PE; x supplied host-transposed
    [D, S] so both operands have the contraction on partitions.
  - Q/K projected to natural [s, d] tiles, RoPE applied with free-dim
    shifted DVE ops, then PE-transposed into QT/KT [d_headpair(128), S].
  - scoresT[k, q] per head via row-paired matmuls (two heads concurrently
    in row-groups 0-1 / 2-3 of the PE array; contraction = dk = 64).
  - softmax without max-subtraction (scores bounded ~|10| for this
    problem); exp on ACT straight out of PSUM; causal masking of diagonal
    blocks via gpsimd affine_select; fully-masked blocks skipped.
  - attn @ V with V' = [V | ones] as stationary (M=65): row 64 accumulates
    the softmax denominator for free. contextT stays unnormalized.
  - normalization: recip(den) broadcast across the 128 head-pair
    partitions with a K=2 indicator matmul, then fused into the PSUM->SBUF
    eviction muls.
  - output projection consumes contextT directly as lhsT (contraction =
    head dims on partitions); per-core result is a [S, D] partial sum.
"""

import json
import os

import numpy as np

# ---------------------------------------------------------------------------
# Workaround: this container's walrus accepts only ONE sync-wait per
# instruction. Hoist every instruction's waits onto single-wait NoOps
# inserted immediately before it (same engine, same program order).
# ---------------------------------------------------------------------------
_PATCHED = False


def _split_multiwait_bir(bir_json: bytes) -> bytes:
    m = json.loads(bir_json)
    ctr = 0
    changed = False
    for f in m.get("functions", []):
        for bl in f.get("blocks", []):
            out = []
            for inst in bl.get("instructions", []):
                si = inst.get("sync_info")
                ow = (si or {}).get("on_wait") or []
                if len(ow) > 1:
                    changed = True
                    for w in ow:
                        ctr += 1
                        out.append({
                            "debug": inst.get("debug", 0),
                            "engine": inst["engine"],
                            "ins": [],
                            "name": f"WSPLIT-{ctr}",
                            "opcode": "NoOp",
                            "outs": [],
                            "sync_info": {"on_update": [], "on_wait": [w]},
                        })
                    si["on_wait"] = []
                out.append(inst)
            if changed:
                bl["instructions"] = out
    if not changed:
        return bir_json
    return json.dumps(m).encode()


def _install_ntff_hook():
    """The agent image's antenv lacks the axon_hooks shim that bass_utils
    imports for trace=True under axon; synthesize it and register the
    ctypes-based NTFF hook from trn_agent_boot (degrades to no-trace if
    anything is missing)."""
    import sys
    import types

    if "antenv.axon_hooks" in sys.modules:
        return
    mod = types.ModuleType("antenv.axon_hooks")
    holder = [None]
    mod.set_axon_ntff_profile_hook = lambda h: holder.__setitem__(0, h)
    mod.get_axon_ntff_profile_hook = lambda: holder[0]
    sys.modules["antenv.axon_hooks"] = mod
    try:
        import antenv
        antenv.axon_hooks = mod
        from trn_agent_boot.trn_boot import _ntff_profile_via_ctypes
        mod.set_axon_ntff_profile_hook(
            _ntff_profile_via_ctypes("/opt/axon/libaxon_pjrt.so"))
    except Exception:
        pass


def _install_patches():
    global _PATCHED
    if _PATCHED:
        return
    import concourse.bass as bass

    orig = bass.Bass.to_json_bytes

    def to_json_bytes_patched(self, *a, **k):
        return _split_multiwait_bir(orig(self, *a, **k))

    bass.Bass.to_json_bytes = to_json_bytes_patched
    _install_ntff_hook()
    _PATCHED = True


# ---------------------------------------------------------------------------
# Problem constants (hardcoded per the harness contract)
# ---------------------------------------------------------------------------
B, S, D = 4, 2048, 1024
H, DK = 16, 64
HG = 8                    # heads per core
DG = HG * DK              # 512: head-group width
N_CORES = 8
ROPE_BASE = 10000.0
P = 128                   # partitions
ST = S // P               # 16 s-tiles
CC = D // P               # 8 contraction chunks for projections
QR = S // 512             # 4 q-ranges of 512
HPAIRS = HG // 2          # 4 head pairs
VSTRIDE = 65              # V columns + ones column


def _build_program(use_bias: bool, phases: int = 3, mm_dt: str = "fp32"):
    import concourse.bass as bass
    import concourse.mybir as mybir
    import concourse.tile as tile
    from concourse.masks import make_identity

    F32 = mybir.dt.float32
    MDT = mybir.dt.float32r if mm_dt == "fp32r" else F32

    def mmcast(ap):
        return ap

    def dcast(ap):
        # DRAM-side view matching MDT-typed SBUF tiles (bit-identical)
        return ap.bitcast(MDT) if MDT is not F32 else ap
    nc = bass.Bass()

    xqT = nc.dram_tensor("xqT", [D, S], F32, kind="ExternalInput")
    xkT = nc.dram_tensor("xkT", [D, S], F32, kind="ExternalInput")
    xvT = nc.dram_tensor("xvT", [D, S], F32, kind="ExternalInput")
    wqT = nc.dram_tensor("wqT", [D, DG], F32, kind="ExternalInput")
    wkT = nc.dram_tensor("wkT", [D, DG], F32, kind="ExternalInput")
    wvT = nc.dram_tensor("wvT", [D, DG], F32, kind="ExternalInput")
    woT = nc.dram_tensor("woT", [DG, D], F32, kind="ExternalInput")
    cos_d = nc.dram_tensor("cos_d", [S, DK], F32, kind="ExternalInput")
    ssg_d = nc.dram_tensor("ssg_d", [S, DK], F32, kind="ExternalInput")
    if use_bias:
        bias_d = nc.dram_tensor("bias_d", [4, DG], F32, kind="ExternalInput")
        ones_d = nc.dram_tensor("ones_d", [1, P], F32, kind="ExternalInput")
    out_d = nc.dram_tensor("out", [S, D], F32, kind="ExternalOutput")

    with tile.TileContext(nc) as tc:
        with tc.tile_pool(name="consts", bufs=1) as consts, \
             tc.tile_pool(name="xT", bufs=8) as xT_pool, \
             tc.tile_pool(name="w", bufs=8) as w_pool, \
             tc.tile_pool(name="nat", bufs=3) as nat_pool, \
             tc.tile_pool(name="qk", bufs=8) as qk_pool, \
             tc.tile_pool(name="vp", bufs=1) as v_pool, \
             tc.tile_pool(name="ctx", bufs=4) as ctx_pool, \
             tc.tile_pool(name="den", bufs=1) as den_pool, \
             tc.tile_pool(name="w512", bufs=6) as work_pool, \
             tc.tile_pool(name="psm", bufs=4, space="PSUM") as ps_main, \
             tc.tile_pool(name="psc", bufs=4, space="PSUM") as ps_ctx:

            ident = consts.tile([P, P], F32)
            make_identity(nc, ident)
            ones1 = consts.tile([1, 64], F32)
            nc.vector.memset(ones1, 1.0)
            # cos/ssign: [S, 64] -> [128, 16*64] (s = st*128 + p)
            cos_sb = consts.tile([P, ST * DK], F32)
            nc.sync.dma_start(out=cos_sb,
                              in_=cos_d.rearrange("(t p) d -> p t d", p=P))
            ssg_sb = consts.tile([P, ST * DK], F32)
            nc.sync.dma_start(out=ssg_sb,
                              in_=ssg_d.rearrange("(t p) d -> p t d", p=P))
            if use_bias:
                bias_sb = consts.tile([4, DG], F32)
                nc.sync.dma_start(out=bias_sb, in_=bias_d[:, :])
                ones_sb = consts.tile([1, P], F32)
                nc.sync.dma_start(out=ones_sb, in_=ones_d[:, :])

            # persistent activations
            qT = [qk_pool.tile([P, S], MDT, tag="qk", name=f"qT{i}") for i in range(HPAIRS)]
            kT = [qk_pool.tile([P, S], MDT, tag="qk", name=f"kT{i}") for i in range(HPAIRS)]
            v_all = v_pool.tile([P, HG * ST * VSTRIDE], MDT)
            # ones columns of V' (single strided broadcast copy)
            ones_col = consts.tile([P, 1], F32)
            nc.vector.memset(ones_col, 1.0)
            ones_bc = bass.AP(tensor=ones_col.tensor, offset=ones_col.offset,
                              ap=[ones_col.ap[0], [0, HG], [0, ST], [0, 1]])
            nc.vector.tensor_copy(
                v_all.rearrange("p (h t c) -> p h t c", h=HG, t=ST)[:, :, :, DK:DK + 1],
                ones_bc)
            ctxT = [ctx_pool.tile([P, S], MDT, tag="ctx", name=f"ctxT{i}") for i in range(HPAIRS)]

            # ---------------- projections + RoPE + transposes --------------
            def cos_bc(st, half):
                # cos/ssign slice [128, 32] broadcast over 8 heads
                src = cos_sb if half is None else ssg_sb
                width = DK if half is None else 32
                off = st * DK + (0 if half in (None, 0) else 32)
                sl = src[:, off:off + width]
                return bass.AP(tensor=sl.tensor, offset=sl.offset,
                               ap=[sl.ap[0], [0, HG], [1, width]])

            for t_i, (x_t, w_t) in enumerate(((xqT, wqT), (xkT, wkT), (xvT, wvT))):
                for sg in range(QR):           # groups of 4 s-tiles
                    xg = [xT_pool.tile([P, 512], MDT, tag="xT", name=f"xg{i}") for i in range(CC)]
                    for cc in range(CC):
                        nc.sync.dma_start(
                            out=xg[cc],
                            in_=dcast(x_t[cc * P:(cc + 1) * P,
                                          sg * 512:(sg + 1) * 512]))
                    if sg == 0:
                        wg = [w_pool.tile([P, DG], MDT, tag="w", name=f"wg{i}") for i in range(CC)]
                        for cc in range(CC):
                            nc.sync.dma_start(
                                out=wg[cc],
                                in_=dcast(w_t[cc * P:(cc + 1) * P, :]))
                    for sti in range(4):
                        st = sg * 4 + sti
                        psum = ps_main.tile([P, DG], F32, tag="ps")
                        if use_bias:
                            nc.tensor.matmul(psum, ones_sb,
                                             bias_sb[t_i:t_i + 1, :],
                                             start=True, stop=False)
                        for cc in range(CC):
                            nc.tensor.matmul(
                                psum, mmcast(xg[cc][:, sti * P:(sti + 1) * P]),
                                mmcast(wg[cc]),
                                start=(cc == 0 and not use_bias),
                                stop=(cc == CC - 1))
                        if t_i < 2:
                            # RoPE: nat = psum*cos ; nat += shift(psum)*ssign
                            nat = nat_pool.tile([P, DG], F32, tag="nat")
                            tmp = work_pool.tile([P, DG], F32, tag="w512")
                            nat4 = nat.rearrange("p (h t d) -> p h t d", h=HG, t=2)
                            tmp4 = tmp.rearrange("p (h t d) -> p h t d", h=HG, t=2)
                            ps4 = psum.rearrange("p (h t d) -> p h t d", h=HG, t=2)
                            nc.vector.tensor_mul(
                                nat.rearrange("p (h d) -> p h d", h=HG),
                                psum.rearrange("p (h d) -> p h d", h=HG),
                                cos_bc(st, None))
                            nc.vector.tensor_mul(tmp4[:, :, 0, :], ps4[:, :, 1, :],
                                                 cos_bc(st, 0))
                            nc.vector.tensor_mul(tmp4[:, :, 1, :], ps4[:, :, 0, :],
                                                 cos_bc(st, 1))
                            nc.vector.tensor_add(nat, nat, tmp)
                            dest = qT if t_i == 0 else kT
                            for hp in range(HPAIRS):
                                pt = ps_ctx.tile([P, P], F32, tag="pc")
                                nc.tensor.transpose(
                                    pt, nat[:, hp * P:(hp + 1) * P], ident)
                                nc.vector.tensor_copy(
                                    dest[hp][:, st * P:(st + 1) * P], pt)
                        else:
                            v4 = v_all.rearrange("p (h t c) -> p h t c",
                                                 h=HG, t=ST)
                            for h in range(HG):
                                nc.vector.tensor_copy(
                                    v4[:, h, st, 0:DK],
                                    psum[:, h * DK:(h + 1) * DK])

            if phases < 2:
                for i in range(4):
                    ot = work_pool.tile([P, 512], F32, tag="w512",
                                        name=f"dump{i}")
                    nc.vector.tensor_copy(ot, qT[i][:, 0:512].bitcast(F32))
                    nc.sync.dma_start(out=out_d[i * P:(i + 1) * P, 0:512], in_=ot)
                return nc
            # ------------- attention + inlined output projection ------------
            # qr-outer so each q-range's output projection follows right
            # after its attention, giving PE dense filler work while ACT
            # works through the exps (keeps HAM warm).
            is_ge = mybir.AluOpType.is_ge
            Exp = mybir.ActivationFunctionType.Exp
            wo = {}
            if phases >= 3:
                for nr in range(2):
                    for dc in range(4):
                        wt = w_pool.tile([P, 512], MDT, tag="w",
                                         name=f"wo{nr}_{dc}")
                        nc.sync.dma_start(
                            out=wt,
                            in_=dcast(woT[dc * P:(dc + 1) * P,
                                          nr * 512:(nr + 1) * 512]))
                        wo[(nr, dc)] = wt
            for qr in range(QR):
                for hp in range(HPAIRS):
                    hA, hB = 2 * hp, 2 * hp + 1
                    pcA = ps_ctx.tile([VSTRIDE, 512], F32, tag="pc")
                    pcB = ps_ctx.tile([VSTRIDE, 512], F32, tag="pc")
                    n_kc = 4 * (qr + 1)
                    for kc in range(n_kc):
                        psA = ps_main.tile([P, 512], F32, tag="ps")
                        psB = ps_main.tile([P, 512], F32, tag="ps")
                        qsl = slice(qr * 512, (qr + 1) * 512)
                        ksl = slice(kc * P, (kc + 1) * P)
                        nc.tensor.matmul(psA, mmcast(kT[hp][0:64, ksl]),
                                         mmcast(qT[hp][0:64, qsl]),
                                         start=True, stop=True, tile_position=(0, 0))
                        nc.tensor.matmul(psB, mmcast(kT[hp][64:128, ksl]),
                                         mmcast(qT[hp][64:128, qsl]),
                                         start=True, stop=True, tile_position=(64, 0))
                        eA = work_pool.tile([P, 512], MDT, tag="w512")
                        eB = work_pool.tile([P, 512], MDT, tag="w512")
                        nc.scalar.activation(out=eA, in_=psA, func=Exp, scale=0.125)
                        nc.scalar.activation(out=eB, in_=psB, func=Exp, scale=0.125)
                        j = kc - 4 * qr
                        if j >= 0:  # diagonal block: keep qq - kk - 128*j >= 0
                            for e in (eA, eB):
                                nc.gpsimd.affine_select(
                                    out=e, in_=e, compare_op=is_ge, fill=0.0,
                                    base=-128 * j, channel_multiplier=-1,
                                    pattern=[[1, 512]])
                        v4 = v_all.rearrange("p (h t c) -> p h t c", h=HG, t=ST)
                        nc.tensor.matmul(pcA, mmcast(v4[:, hA, kc, :]), mmcast(eA),
                                         start=(kc == 0), stop=(kc == n_kc - 1))
                        nc.tensor.matmul(pcB, mmcast(v4[:, hB, kc, :]), mmcast(eB),
                                         start=(kc == 0), stop=(kc == n_kc - 1))
                    qsl = slice(qr * 512, (qr + 1) * 512)
                    denA = den_pool.tile([1, 512], F32, tag="rec", bufs=4,
                                         name="denA")
                    denB = den_pool.tile([1, 512], F32, tag="rec", bufs=4,
                                         name="denB")
                    nc.vector.tensor_copy(denA, pcA[64:65, :])
                    nc.vector.tensor_copy(denB, pcB[64:65, :])
                    pbc = ps_main.tile([P, 512], F32, tag="ps")
                    nc.tensor.matmul(pbc[0:64, :], ones1, denA,
                                     start=True, stop=True, tile_position=(0, 0),
                                     skip_group_check=True)
                    nc.tensor.matmul(pbc[64:128, :], ones1, denB,
                                     start=True, stop=True, tile_position=(0, 64),
                                     skip_group_check=True)
                    rbc = work_pool.tile([P, 512], F32, tag="w512")
                    nc.vector.reciprocal(out=rbc, in_=pbc)
                    nc.vector.tensor_mul(ctxT[hp][0:64, qsl], pcA[0:64, :],
                                         rbc[0:64, :])
                    nc.vector.tensor_mul(ctxT[hp][64:128, qsl], pcB[0:64, :],
                                         rbc[64:128, :])

                if phases >= 3:
                    for sti in range(4):
                        st = qr * 4 + sti
                        for nr in range(2):
                            po = ps_main.tile([P, 512], F32, tag="ps")
                            for dc in range(4):
                                nc.tensor.matmul(
                                    po, mmcast(ctxT[dc][:, st * P:(st + 1) * P]),
                                    mmcast(wo[(nr, dc)]),
                                    start=(dc == 0), stop=(dc == 3))
                            ot = work_pool.tile([P, 512], F32, tag="w512")
                            nc.vector.tensor_copy(ot, po)
                            nc.sync.dma_start(
                                out=out_d[st * P:(st + 1) * P,
                                          nr * 512:(nr + 1) * 512],
                                in_=ot)

            if phases < 3:
                for i in range(4):
                    ot = work_pool.tile([P, 512], F32, tag="w512",
                                        name=f"dump{i}")
                    nc.vector.tensor_copy(ot, ctxT[i][:, 0:512].bitcast(F32))
                    nc.sync.dma_start(out=out_d[i * P:(i + 1) * P, 0:512], in_=ot)
    return nc


def _build_program_v3(use_bias: bool, mm_dt: str = "fp32r"):
    """Interleaved emission: projection and output-projection PE work is
    round-robined into the attention instruction stream so the in-order
    PE has filler work while ACT computes exps (keeps HAM warm)."""
    from collections import deque

    import concourse.bass as bass
    import concourse.mybir as mybir
    import concourse.tile as tile
    from concourse.masks import make_identity

    F32 = mybir.dt.float32
    MDT = mybir.dt.float32r if mm_dt == "fp32r" else F32

    def dcast(ap):
        return ap.bitcast(MDT) if MDT is not F32 else ap

    nc = bass.Bass()
    xs = {t: nc.dram_tensor(f"x{t}T", [D, S], F32, kind="ExternalInput")
          for t in "qkv"}
    ws = {t: nc.dram_tensor(f"w{t}T", [D, DG], F32, kind="ExternalInput")
          for t in "qkv"}
    woT = nc.dram_tensor("woT", [DG, D], F32, kind="ExternalInput")
    cos_d = nc.dram_tensor("cos_d", [S, DK], F32, kind="ExternalInput")
    ssg_d = nc.dram_tensor("ssg_d", [S, DK], F32, kind="ExternalInput")
    if use_bias:
        bias_d = nc.dram_tensor("bias_d", [4, DG], F32, kind="ExternalInput")
        ones_d = nc.dram_tensor("ones_d", [1, P], F32, kind="ExternalInput")
    out_d = nc.dram_tensor("out", [S, D], F32, kind="ExternalOutput")

    with tile.TileContext(nc) as tc:
        with tc.tile_pool(name="consts", bufs=1) as consts, \
             tc.tile_pool(name="xT", bufs=8) as xT_pool, \
             tc.tile_pool(name="w", bufs=32) as w_pool, \
             tc.tile_pool(name="nat", bufs=2) as nat_pool, \
             tc.tile_pool(name="kt", bufs=4) as kt_pool, \
             tc.tile_pool(name="qt", bufs=8) as qt_pool, \
             tc.tile_pool(name="vp", bufs=1) as v_pool, \
             tc.tile_pool(name="ctx", bufs=8) as ctx_pool, \
             tc.tile_pool(name="den", bufs=1) as den_pool, \
             tc.tile_pool(name="w512", bufs=4) as work_pool, \
             tc.tile_pool(name="psm", bufs=4, space="PSUM") as ps_main, \
             tc.tile_pool(name="psb", bufs=1, space="PSUM") as ps_bc, \
             tc.tile_pool(name="psc", bufs=3, space="PSUM") as ps_ctx:

            ident = consts.tile([P, P], F32)
            make_identity(nc, ident)
            ones1 = consts.tile([1, 64], F32)
            nc.vector.memset(ones1, 1.0)
            cos_sb = consts.tile([P, ST * DK], F32)
            nc.sync.dma_start(out=cos_sb,
                              in_=cos_d.rearrange("(t p) d -> p t d", p=P))
            ssg_sb = consts.tile([P, ST * DK], F32)
            nc.sync.dma_start(out=ssg_sb,
                              in_=ssg_d.rearrange("(t p) d -> p t d", p=P))
            if use_bias:
                bias_sb = consts.tile([4, DG], F32)
                nc.sync.dma_start(out=bias_sb, in_=bias_d[:, :])
                ones_sb = consts.tile([1, P], F32)
                nc.sync.dma_start(out=ones_sb, in_=ones_d[:, :])

            kT = [kt_pool.tile([P, S], MDT, tag="kt", name=f"kT{i}")
                  for i in range(HPAIRS)]
            v_all = v_pool.tile([P, HG * ST * VSTRIDE], MDT)
            ones_col = consts.tile([P, 1], F32)
            nc.vector.memset(ones_col, 1.0)
            ones_bc = bass.AP(tensor=ones_col.tensor, offset=ones_col.offset,
                              ap=[ones_col.ap[0], [0, HG], [0, ST], [0, 1]])
            v4 = v_all.rearrange("p (h t c) -> p h t c", h=HG, t=ST)
            nc.vector.tensor_copy(v4[:, :, :, DK:DK + 1], ones_bc)

            # all weights resident
            wg = {}
            for ti, t in enumerate("qkv"):
                for cc in range(CC):
                    wt = w_pool.tile([P, DG], MDT, tag="w", name=f"w{t}{cc}")
                    nc.sync.dma_start(out=wt,
                                      in_=dcast(ws[t][cc * P:(cc + 1) * P, :]))
                    wg[(t, cc)] = wt
            wo = {}
            for nr in range(2):
                for dc in range(4):
                    wt = w_pool.tile([P, 512], MDT, tag="w",
                                     name=f"wo{nr}_{dc}")
                    nc.sync.dma_start(
                        out=wt, in_=dcast(woT[dc * P:(dc + 1) * P,
                                               nr * 512:(nr + 1) * 512]))
                    wo[(nr, dc)] = wt

            qts = {}   # (sg, hp) -> [128, 512] MDT
            ctxs = {}  # (qr, hp) -> [128, 512] MDT
            xgs = {}   # (t, sg) -> chunk list
            pending_nat = []

            def flush_transposes():
                while pending_nat:
                    ti, sg, sti, st, nat = pending_nat.pop(0)
                    for hp in range(HPAIRS):
                        pt = ps_main.tile([P, P], F32, tag="ps", name="pt")
                        nc.tensor.transpose(pt, nat[:, hp * P:(hp + 1) * P],
                                            ident)
                        if ti == 0:
                            nc.vector.tensor_copy(
                                qts[(sg, hp)][:, sti * P:(sti + 1) * P], pt)
                        else:
                            nc.vector.tensor_copy(
                                kT[hp][:, st * P:(st + 1) * P], pt)

            def cos_bc(st, half):
                src = cos_sb if half is None else ssg_sb
                width = DK if half is None else 32
                off = st * DK + (0 if half in (None, 0) else 32)
                sl = src[:, off:off + width]
                return bass.AP(tensor=sl.tensor, offset=sl.offset,
                               ap=[sl.ap[0], [0, HG], [1, width]])

            def emit_proj_dma(t, sg):
                xg = [xT_pool.tile([P, 512], MDT, tag="xT",
                                   name=f"x{t}{sg}_{i}") for i in range(CC)]
                for cc in range(CC):
                    nc.sync.dma_start(
                        out=xg[cc],
                        in_=dcast(xs[t][cc * P:(cc + 1) * P,
                                        sg * 512:(sg + 1) * 512]))
                xgs[(t, sg)] = xg

            def emit_proj_unit(ti, t, sg, sti):
                st = sg * 4 + sti
                if sti == 0 and ti == 0:
                    for hp in range(HPAIRS):
                        qts[(sg, hp)] = qt_pool.tile(
                            [P, 512], MDT, tag="qt", name=f"qt{sg}_{hp}")
                xg = xgs[(t, sg)]
                psum = ps_main.tile([P, DG], F32, tag="ps")
                if use_bias:
                    nc.tensor.matmul(psum, ones_sb, bias_sb[ti:ti + 1, :],
                                     start=True, stop=False)
                for cc in range(CC):
                    nc.tensor.matmul(psum, xg[cc][:, sti * P:(sti + 1) * P],
                                     wg[(t, cc)],
                                     start=(cc == 0 and not use_bias),
                                     stop=(cc == CC - 1))
                if ti < 2:
                    flush_transposes()
                    nat = nat_pool.tile([P, DG], F32, tag="nat")
                    tmp = work_pool.tile([P, DG], F32, tag="w512")
                    tmp4 = tmp.rearrange("p (h t d) -> p h t d", h=HG, t=2)
                    ps4 = psum.rearrange("p (h t d) -> p h t d", h=HG, t=2)
                    nc.vector.tensor_mul(
                        nat.rearrange("p (h d) -> p h d", h=HG),
                        psum.rearrange("p (h d) -> p h d", h=HG),
                        cos_bc(st, None))
                    nc.vector.tensor_mul(tmp4[:, :, 0, :], ps4[:, :, 1, :],
                                         cos_bc(st, 0))
                    nc.vector.tensor_mul(tmp4[:, :, 1, :], ps4[:, :, 0, :],
                                         cos_bc(st, 1))
                    nc.vector.tensor_add(nat, nat, tmp)
                    # transposes run one unit later (PE meets them after the
                    # in-order DVE has drained this unit's RoPE chain)
                    pending_nat.append((ti, sg, sti, st, nat))
                else:
                    for h in range(HG):
                        nc.vector.tensor_copy(v4[:, h, st, 0:DK],
                                              psum[:, h * DK:(h + 1) * DK])

            def emit_outproj_unit(qr, sti, nr):
                st = qr * 4 + sti
                po = ps_main.tile([P, 512], F32, tag="ps")
                for dc in range(4):
                    nc.tensor.matmul(po, ctxs[(qr, dc)][:, sti * P:(sti + 1) * P],
                                     wo[(nr, dc)], start=(dc == 0),
                                     stop=(dc == 3))
                ot = work_pool.tile([P, 512], F32, tag="w512")
                nc.scalar.copy(ot, po)
                nc.sync.dma_start(
                    out=out_d[st * P:(st + 1) * P, nr * 512:(nr + 1) * 512],
                    in_=ot)

            is_ge = mybir.AluOpType.is_ge
            Exp = mybir.ActivationFunctionType.Exp

            # prologue: projections for s-group 0, prefetch s-group 1
            for ti, t in enumerate("qkv"):
                emit_proj_dma(t, 0)
                for sti in range(4):
                    emit_proj_unit(ti, t, 0, sti)
            for t in "qkv":
                emit_proj_dma(t, 1)

            from functools import partial
            for qr in range(QR):
                flush_transposes()
                fillers = deque()
                if qr + 1 < QR:
                    for ti, t in enumerate("qkv"):
                        for sti in range(4):
                            fillers.append(
                                partial(emit_proj_unit, ti, t, qr + 1, sti))
                if qr + 2 < QR:
                    # prefetch next-next s-group's x chunks well before use
                    for t in "qkv":
                        fillers.append(partial(emit_proj_dma, t, qr + 2))
                if qr >= 1:
                    for sti in range(4):
                        for nr in range(2):
                            fillers.append(
                                partial(emit_outproj_unit, qr - 1, sti, nr))
                n_slots = 16 * (qr + 1)
                # +HPAIRS: the deferred per-head-pair normalizes are appended
                # while the loop runs; reserve pace slots so they drain
                # interleaved instead of bursting at the qr boundary
                pace = max(1, n_slots // (len(fillers) + HPAIRS))
                cnt = 0
                for hp in range(HPAIRS):
                    hA, hB = 2 * hp, 2 * hp + 1
                    pcA = ps_ctx.tile([VSTRIDE, 512], F32, tag="pc")
                    pcB = ps_ctx.tile([VSTRIDE, 512], F32, tag="pc")
                    n_kc = 4 * (qr + 1)
                    for kc in range(n_kc):
                        psA = ps_main.tile([P, 512], F32, tag="ps")
                        psB = ps_main.tile([P, 512], F32, tag="ps")
                        ksl = slice(kc * P, (kc + 1) * P)
                        nc.tensor.matmul(psA, kT[hp][0:64, ksl],
                                         qts[(qr, hp)][0:64, :],
                                         start=True, stop=True,
                                         tile_position=(0, 0))
                        nc.tensor.matmul(psB, kT[hp][64:128, ksl],
                                         qts[(qr, hp)][64:128, :],
                                         start=True, stop=True,
                                         tile_position=(64, 0))
                        eA = work_pool.tile([P, 512], MDT, tag="w512")
                        eB = work_pool.tile([P, 512], MDT, tag="w512")
                        nc.scalar.activation(out=eA, in_=psA, func=Exp,
                                             scale=0.125)
                        nc.scalar.activation(out=eB, in_=psB, func=Exp,
                                             scale=0.125)
                        j = kc - 4 * qr
                        if j >= 0:
                            for e in (eA, eB):
                                nc.gpsimd.affine_select(
                                    out=e, in_=e, compare_op=is_ge, fill=0.0,
                                    base=-128 * j, channel_multiplier=-1,
                                    pattern=[[1, 512]])
                        nc.tensor.matmul(pcA, v4[:, hA, kc, :], eA,
                                         start=(kc == 0), stop=(kc == n_kc - 1))
                        nc.tensor.matmul(pcB, v4[:, hB, kc, :], eB,
                                         start=(kc == 0), stop=(kc == n_kc - 1))
                        cnt += 1
                        if cnt % pace == 0 and fillers:
                            fillers.popleft()()
                    for hp2 in (hA, hB):
                        pass
                    # Quick-release the ctx PSUM banks: copy out rows
                    # unnormalized, then normalize in SBUF off the PE
                    # critical path (the 3.4us DVE reciprocal otherwise
                    # stalls the next head-pair long enough to re-throttle
                    # the PE clock).
                    denA = den_pool.tile([1, 512], F32, tag="rec", bufs=3,
                                         name="denA")
                    denB = den_pool.tile([1, 512], F32, tag="rec", bufs=3,
                                         name="denB")
                    nc.scalar.copy(denA, pcA[64:65, :])
                    nc.scalar.copy(denB, pcB[64:65, :])
                    ctx = ctx_pool.tile([P, 512], MDT, tag="ctx",
                                        name=f"ctx{qr}_{hp}")
                    ctxs[(qr, hp)] = ctx
                    nc.scalar.copy(ctx[0:64, :], pcA[0:64, :])
                    nc.scalar.copy(ctx[64:128, :], pcB[0:64, :])

                    def emit_normalize(ctx=ctx, denA=denA, denB=denB):
                        # deferred: emitted a few attention slots later so
                        # the PE/DVE never stall at the head-pair boundary
                        pbc = ps_bc.tile([P, 512], F32, tag="pbc",
                                         name="pbc")
                        nc.tensor.matmul(pbc[0:64, :], ones1, denA,
                                         start=True, stop=True,
                                         tile_position=(0, 0),
                                         skip_group_check=True)
                        nc.tensor.matmul(pbc[64:128, :], ones1, denB,
                                         start=True, stop=True,
                                         tile_position=(0, 64),
                                         skip_group_check=True)
                        rbc = work_pool.tile([P, 512], F32, tag="rbc",
                                             bufs=2, name="rbc")
                        nc.vector.reciprocal(out=rbc, in_=pbc)
                        nc.gpsimd.tensor_mul(ctx[0:64, :], ctx[0:64, :],
                                             rbc[0:64, :])
                        nc.gpsimd.tensor_mul(ctx[64:128, :], ctx[64:128, :],
                                             rbc[64:128, :])

                    fillers.append(emit_normalize)
                while fillers:
                    fillers.popleft()()
            for sti in range(4):
                for nr in range(2):
                    emit_outproj_unit(QR - 1, sti, nr)
    return nc


_PROG_CACHE = {}


def _get_program(use_bias: bool):
    mm_dt = os.environ.get("KERNEL_MM_DT", "fp32r")
    key = (use_bias, mm_dt)
    if key not in _PROG_CACHE:
        if os.environ.get("KERNEL_V3", "1") == "1":
            _PROG_CACHE[key] = _build_program_v3(use_bias, mm_dt=mm_dt)
        else:
            _PROG_CACHE[key] = _build_program(use_bias, mm_dt=mm_dt)
    return _PROG_CACHE[key]


def _rope_tables():
    inv = 1.0 / (ROPE_BASE ** (np.arange(0, DK, 2, dtype=np.float32) / DK))
    t = np.arange(S, dtype=np.float32)
    fr = t[:, None] * inv[None, :]                      # [S, 32]
    emb = np.concatenate([fr, fr], axis=-1)             # [S, 64]
    cos = np.cos(emb).astype(np.float32)
    sin = np.sin(emb).astype(np.float32)
    ssg = sin.copy()
    ssg[:, :32] = -sin[:, :32]
    return cos, ssg


def kernel(query, key, value, W_q, b_q, W_k, b_k, W_v, b_v, W_o, b_o):
    _install_patches()
    from concourse.bass_utils import run_bass_kernel_spmd

    query = np.asarray(query, dtype=np.float32)
    key = np.asarray(key, dtype=np.float32)
    value = np.asarray(value, dtype=np.float32)
    W_q, W_k, W_v, W_o = (np.asarray(w, dtype=np.float32)
                          for w in (W_q, W_k, W_v, W_o))
    b_q, b_k, b_v, b_o = (np.asarray(b, dtype=np.float32)
                          for b in (b_q, b_k, b_v, b_o))

    use_bias = bool(np.any(b_q) or np.any(b_k) or np.any(b_v))
    nc = _get_program(use_bias)

    cos, ssg = _rope_tables()

    in_maps = []
    for c in range(N_CORES):
        b, g = divmod(c, 2)
        gs = slice(g * DG, (g + 1) * DG)
        m = {
            "xqT": np.ascontiguousarray(query[b].T),
            "xkT": np.ascontiguousarray(key[b].T),
            "xvT": np.ascontiguousarray(value[b].T),
            "wqT": np.ascontiguousarray(W_q[gs, :].T),
            "wkT": np.ascontiguousarray(W_k[gs, :].T),
            "wvT": np.ascontiguousarray(W_v[gs, :].T),
            "woT": np.ascontiguousarray(W_o[:, gs].T),
            "cos_d": cos,
            "ssg_d": ssg,
        }
        if use_bias:
            m["bias_d"] = np.stack([b_q[gs], b_k[gs], b_v[gs],
                                    np.zeros(DG, np.float32)])
            m["ones_d"] = np.ones((1, P), np.float32)
        in_maps.append(m)

    trace = bool(int(os.environ.get("KERNEL_TRACE", "0")))
    trace_cores = None
    if trace:
        tc_env = os.environ.get("KERNEL_TRACE_CORES", "")
        trace_cores = ([int(x) for x in tc_env.split(",") if x != ""]
                       if tc_env else list(range(N_CORES)))
    try:
        res = run_bass_kernel_spmd(nc, in_maps, core_ids=list(range(N_CORES)),
                                   trace=trace, trace_cores=trace_cores)
    except Exception:
        if not trace:
            raise
        res = run_bass_kernel_spmd(nc, in_maps, core_ids=list(range(N_CORES)),
                                   trace=False)
    kernel._last_results = res

    out = np.empty((B, S, D), np.float32)
    for b in range(B):
        out[b] = res.results[2 * b]["out"] + res.results[2 * b + 1]["out"] + b_o
    return out



# revision 6
# speedup vs baseline: 1.0721x; 1.0721x over previous
"""Multi-head attention (RoPE, causal) Trainium2 kernel, 8-way sharded.

Sharding: core c => batch b = c//2, head-group g = c%2 (8 of 16 heads).
Each core computes Q/K/V projections for its (b, g), RoPE, causal
attention over its 8 heads, and the row-slice of the output projection.
Host sums the two partial output projections per batch and adds b_o.

Per-core dataflow (fp32 storage; matmuls in float32r — single-pass
reduced-precision fp32, ~1.7e-4 per-matmul rel err, 2.7x the throughput
of full fp32's two half-speed passes):
  - projections contract over model dim via PE; x supplied host-transposed
    [D, S] so both operands have the contraction on partitions.
  - Q/K projected to natural [s, d] tiles, RoPE applied with free-dim
    shifted DVE ops, then PE-transposed into QT/KT [d_headpair(128), S].
  - scoresT[k, q] per head via row-paired matmuls (two heads concurrently
    in row-groups 0-1 / 2-3 of the PE array; contraction = dk = 64).
  - softmax without max-subtraction (scores bounded ~|10| for this
    problem); exp on ACT straight out of PSUM; causal masking of diagonal
    blocks via gpsimd affine_select; fully-masked blocks skipped.
  - attn @ V with V' = [V | ones] as stationary (M=65): row 64 accumulates
    the softmax denominator for free. contextT stays unnormalized.
  - normalization: recip(den) broadcast across the 128 head-pair
    partitions with a K=2 indicator matmul, then fused into the PSUM->SBUF
    eviction muls.
  - output projection consumes contextT directly as lhsT (contraction =
    head dims on partitions); per-core result is a [S, D] partial sum.
"""

import json
import os

import numpy as np

# ---------------------------------------------------------------------------
# Workaround: this container's walrus accepts only ONE sync-wait per
# instruction. Hoist every instruction's waits onto single-wait NoOps
# inserted immediately before it (same engine, same program order).
# ---------------------------------------------------------------------------
_PATCHED = False


def _split_multiwait_bir(bir_json: bytes) -> bytes:
    m = json.loads(bir_json)
    ctr = 0
    changed = False
    for f in m.get("functions", []):
        for bl in f.get("blocks", []):
            out = []
            for inst in bl.get("instructions", []):
                si = inst.get("sync_info")
                ow = (si or {}).get("on_wait") or []
                if len(ow) > 1:
                    changed = True
                    for w in ow:
                        ctr += 1
                        out.append({
                            "debug": inst.get("debug", 0),
                            "engine": inst["engine"],
                            "ins": [],
                            "name": f"WSPLIT-{ctr}",
                            "opcode": "NoOp",
                            "outs": [],
                            "sync_info": {"on_update": [], "on_wait": [w]},
                        })
                    si["on_wait"] = []
                out.append(inst)
            if changed:
                bl["instructions"] = out
    if not changed:
        return bir_json
    return json.dumps(m).encode()


def _install_ntff_hook():
    """The agent image's antenv lacks the axon_hooks shim that bass_utils
    imports for trace=True under axon; synthesize it and register the
    ctypes-based NTFF hook from trn_agent_boot (degrades to no-trace if
    anything is missing)."""
    import sys
    import types

    if "antenv.axon_hooks" in sys.modules:
        return
    mod = types.ModuleType("antenv.axon_hooks")
    holder = [None]
    mod.set_axon_ntff_profile_hook = lambda h: holder.__setitem__(0, h)
    mod.get_axon_ntff_profile_hook = lambda: holder[0]
    sys.modules["antenv.axon_hooks"] = mod
    try:
        import antenv
        antenv.axon_hooks = mod
        from trn_agent_boot.trn_boot import _ntff_profile_via_ctypes
        mod.set_axon_ntff_profile_hook(
            _ntff_profile_via_ctypes("/opt/axon/libaxon_pjrt.so"))
    except Exception:
        pass


def _install_patches():
    global _PATCHED
    if _PATCHED:
        return
    import concourse.bass as bass

    orig = bass.Bass.to_json_bytes

    def to_json_bytes_patched(self, *a, **k):
        return _split_multiwait_bir(orig(self, *a, **k))

    bass.Bass.to_json_bytes = to_json_bytes_patched
    _install_ntff_hook()
    _PATCHED = True


# ---------------------------------------------------------------------------
# Problem constants (hardcoded per the harness contract)
# ---------------------------------------------------------------------------
B, S, D = 4, 2048, 1024
H, DK = 16, 64
HG = 8                    # heads per core
DG = HG * DK              # 512: head-group width
N_CORES = 8
ROPE_BASE = 10000.0
P = 128                   # partitions
ST = S // P               # 16 s-tiles
CC = D // P               # 8 contraction chunks for projections
QR = S // 512             # 4 q-ranges of 512
HPAIRS = HG // 2          # 4 head pairs
VSTRIDE = 65              # V columns + ones column


def _build_program(use_bias: bool, phases: int = 3, mm_dt: str = "fp32"):
    import concourse.bass as bass
    import concourse.mybir as mybir
    import concourse.tile as tile
    from concourse.masks import make_identity

    F32 = mybir.dt.float32
    MDT = mybir.dt.float32r if mm_dt == "fp32r" else F32

    def mmcast(ap):
        return ap

    def dcast(ap):
        # DRAM-side view matching MDT-typed SBUF tiles (bit-identical)
        return ap.bitcast(MDT) if MDT is not F32 else ap
    nc = bass.Bass()

    xqT = nc.dram_tensor("xqT", [D, S], F32, kind="ExternalInput")
    xkT = nc.dram_tensor("xkT", [D, S], F32, kind="ExternalInput")
    xvT = nc.dram_tensor("xvT", [D, S], F32, kind="ExternalInput")
    wqT = nc.dram_tensor("wqT", [D, DG], F32, kind="ExternalInput")
    wkT = nc.dram_tensor("wkT", [D, DG], F32, kind="ExternalInput")
    wvT = nc.dram_tensor("wvT", [D, DG], F32, kind="ExternalInput")
    woT = nc.dram_tensor("woT", [DG, D], F32, kind="ExternalInput")
    cos_d = nc.dram_tensor("cos_d", [S, DK], F32, kind="ExternalInput")
    ssg_d = nc.dram_tensor("ssg_d", [S, DK], F32, kind="ExternalInput")
    if use_bias:
        bias_d = nc.dram_tensor("bias_d", [4, DG], F32, kind="ExternalInput")
        ones_d = nc.dram_tensor("ones_d", [1, P], F32, kind="ExternalInput")
    out_d = nc.dram_tensor("out", [S, D], F32, kind="ExternalOutput")

    with tile.TileContext(nc) as tc:
        with tc.tile_pool(name="consts", bufs=1) as consts, \
             tc.tile_pool(name="xT", bufs=8) as xT_pool, \
             tc.tile_pool(name="w", bufs=8) as w_pool, \
             tc.tile_pool(name="nat", bufs=3) as nat_pool, \
             tc.tile_pool(name="qk", bufs=8) as qk_pool, \
             tc.tile_pool(name="vp", bufs=1) as v_pool, \
             tc.tile_pool(name="ctx", bufs=4) as ctx_pool, \
             tc.tile_pool(name="den", bufs=1) as den_pool, \
             tc.tile_pool(name="w512", bufs=6) as work_pool, \
             tc.tile_pool(name="psm", bufs=4, space="PSUM") as ps_main, \
             tc.tile_pool(name="psc", bufs=4, space="PSUM") as ps_ctx:

            ident = consts.tile([P, P], F32)
            make_identity(nc, ident)
            ones1 = consts.tile([1, 64], F32)
            nc.vector.memset(ones1, 1.0)
            # cos/ssign: [S, 64] -> [128, 16*64] (s = st*128 + p)
            cos_sb = consts.tile([P, ST * DK], F32)
            nc.sync.dma_start(out=cos_sb,
                              in_=cos_d.rearrange("(t p) d -> p t d", p=P))
            ssg_sb = consts.tile([P, ST * DK], F32)
            nc.sync.dma_start(out=ssg_sb,
                              in_=ssg_d.rearrange("(t p) d -> p t d", p=P))
            if use_bias:
                bias_sb = consts.tile([4, DG], F32)
                nc.sync.dma_start(out=bias_sb, in_=bias_d[:, :])
                ones_sb = consts.tile([1, P], F32)
                nc.sync.dma_start(out=ones_sb, in_=ones_d[:, :])

            # persistent activations
            qT = [qk_pool.tile([P, S], MDT, tag="qk", name=f"qT{i}") for i in range(HPAIRS)]
            kT = [qk_pool.tile([P, S], MDT, tag="qk", name=f"kT{i}") for i in range(HPAIRS)]
            v_all = v_pool.tile([P, HG * ST * VSTRIDE], MDT)
            # ones columns of V' (single strided broadcast copy)
            ones_col = consts.tile([P, 1], F32)
            nc.vector.memset(ones_col, 1.0)
            ones_bc = bass.AP(tensor=ones_col.tensor, offset=ones_col.offset,
                              ap=[ones_col.ap[0], [0, HG], [0, ST], [0, 1]])
            nc.vector.tensor_copy(
                v_all.rearrange("p (h t c) -> p h t c", h=HG, t=ST)[:, :, :, DK:DK + 1],
                ones_bc)
            ctxT = [ctx_pool.tile([P, S], MDT, tag="ctx", name=f"ctxT{i}") for i in range(HPAIRS)]

            # ---------------- projections + RoPE + transposes --------------
            def cos_bc(st, half):
                # cos/ssign slice [128, 32] broadcast over 8 heads
                src = cos_sb if half is None else ssg_sb
                width = DK if half is None else 32
                off = st * DK + (0 if half in (None, 0) else 32)
                sl = src[:, off:off + width]
                return bass.AP(tensor=sl.tensor, offset=sl.offset,
                               ap=[sl.ap[0], [0, HG], [1, width]])

            for t_i, (x_t, w_t) in enumerate(((xqT, wqT), (xkT, wkT), (xvT, wvT))):
                for sg in range(QR):           # groups of 4 s-tiles
                    xg = [xT_pool.tile([P, 512], MDT, tag="xT", name=f"xg{i}") for i in range(CC)]
                    for cc in range(CC):
                        nc.sync.dma_start(
                            out=xg[cc],
                            in_=dcast(x_t[cc * P:(cc + 1) * P,
                                          sg * 512:(sg + 1) * 512]))
                    if sg == 0:
                        wg = [w_pool.tile([P, DG], MDT, tag="w", name=f"wg{i}") for i in range(CC)]
                        for cc in range(CC):
                            nc.sync.dma_start(
                                out=wg[cc],
                                in_=dcast(w_t[cc * P:(cc + 1) * P, :]))
                    for sti in range(4):
                        st = sg * 4 + sti
                        psum = ps_main.tile([P, DG], F32, tag="ps")
                        if use_bias:
                            nc.tensor.matmul(psum, ones_sb,
                                             bias_sb[t_i:t_i + 1, :],
                                             start=True, stop=False)
                        for cc in range(CC):
                            nc.tensor.matmul(
                                psum, mmcast(xg[cc][:, sti * P:(sti + 1) * P]),
                                mmcast(wg[cc]),
                                start=(cc == 0 and not use_bias),
                                stop=(cc == CC - 1))
                        if t_i < 2:
                            # RoPE: nat = psum*cos ; nat += shift(psum)*ssign
                            nat = nat_pool.tile([P, DG], F32, tag="nat")
                            tmp = work_pool.tile([P, DG], F32, tag="w512")
                            nat4 = nat.rearrange("p (h t d) -> p h t d", h=HG, t=2)
                            tmp4 = tmp.rearrange("p (h t d) -> p h t d", h=HG, t=2)
                            ps4 = psum.rearrange("p (h t d) -> p h t d", h=HG, t=2)
                            nc.vector.tensor_mul(
                                nat.rearrange("p (h d) -> p h d", h=HG),
                                psum.rearrange("p (h d) -> p h d", h=HG),
                                cos_bc(st, None))
                            nc.vector.tensor_mul(tmp4[:, :, 0, :], ps4[:, :, 1, :],
                                                 cos_bc(st, 0))
                            nc.vector.tensor_mul(tmp4[:, :, 1, :], ps4[:, :, 0, :],
                                                 cos_bc(st, 1))
                            nc.vector.tensor_add(nat, nat, tmp)
                            dest = qT if t_i == 0 else kT
                            for hp in range(HPAIRS):
                                pt = ps_ctx.tile([P, P], F32, tag="pc")
                                nc.tensor.transpose(
                                    pt, nat[:, hp * P:(hp + 1) * P], ident)
                                nc.vector.tensor_copy(
                                    dest[hp][:, st * P:(st + 1) * P], pt)
                        else:
                            v4 = v_all.rearrange("p (h t c) -> p h t c",
                                                 h=HG, t=ST)
                            for h in range(HG):
                                nc.vector.tensor_copy(
                                    v4[:, h, st, 0:DK],
                                    psum[:, h * DK:(h + 1) * DK])

            if phases < 2:
                for i in range(4):
                    ot = work_pool.tile([P, 512], F32, tag="w512",
                                        name=f"dump{i}")
                    nc.vector.tensor_copy(ot, qT[i][:, 0:512].bitcast(F32))
                    nc.sync.dma_start(out=out_d[i * P:(i + 1) * P, 0:512], in_=ot)
                return nc
            # ------------- attention + inlined output projection ------------
            # qr-outer so each q-range's output projection follows right
            # after its attention, giving PE dense filler work while ACT
            # works through the exps (keeps HAM warm).
            is_ge = mybir.AluOpType.is_ge
            Exp = mybir.ActivationFunctionType.Exp
            wo = {}
            if phases >= 3:
                for nr in range(2):
                    for dc in range(4):
                        wt = w_pool.tile([P, 512], MDT, tag="w",
                                         name=f"wo{nr}_{dc}")
                        nc.sync.dma_start(
                            out=wt,
                            in_=dcast(woT[dc * P:(dc + 1) * P,
                                          nr * 512:(nr + 1) * 512]))
                        wo[(nr, dc)] = wt
            for qr in range(QR):
                for hp in range(HPAIRS):
                    hA, hB = 2 * hp, 2 * hp + 1
                    pcA = ps_ctx.tile([VSTRIDE, 512], F32, tag="pc")
                    pcB = ps_ctx.tile([VSTRIDE, 512], F32, tag="pc")
                    n_kc = 4 * (qr + 1)
                    for kc in range(n_kc):
                        psA = ps_main.tile([P, 512], F32, tag="ps")
                        psB = ps_main.tile([P, 512], F32, tag="ps")
                        qsl = slice(qr * 512, (qr + 1) * 512)
                        ksl = slice(kc * P, (kc + 1) * P)
                        nc.tensor.matmul(psA, mmcast(kT[hp][0:64, ksl]),
                                         mmcast(qT[hp][0:64, qsl]),
                                         start=True, stop=True, tile_position=(0, 0))
                        nc.tensor.matmul(psB, mmcast(kT[hp][64:128, ksl]),
                                         mmcast(qT[hp][64:128, qsl]),
                                         start=True, stop=True, tile_position=(64, 0))
                        eA = work_pool.tile([P, 512], MDT, tag="w512")
                        eB = work_pool.tile([P, 512], MDT, tag="w512")
                        nc.scalar.activation(out=eA, in_=psA, func=Exp, scale=0.125)
                        nc.scalar.activation(out=eB, in_=psB, func=Exp, scale=0.125)
                        j = kc - 4 * qr
                        if j >= 0:  # diagonal block: keep qq - kk - 128*j >= 0
                            for e in (eA, eB):
                                nc.gpsimd.affine_select(
                                    out=e, in_=e, compare_op=is_ge, fill=0.0,
                                    base=-128 * j, channel_multiplier=-1,
                                    pattern=[[1, 512]])
                        v4 = v_all.rearrange("p (h t c) -> p h t c", h=HG, t=ST)
                        nc.tensor.matmul(pcA, mmcast(v4[:, hA, kc, :]), mmcast(eA),
                                         start=(kc == 0), stop=(kc == n_kc - 1))
                        nc.tensor.matmul(pcB, mmcast(v4[:, hB, kc, :]), mmcast(eB),
                                         start=(kc == 0), stop=(kc == n_kc - 1))
                    qsl = slice(qr * 512, (qr + 1) * 512)
                    denA = den_pool.tile([1, 512], F32, tag="rec", bufs=4,
                                         name="denA")
                    denB = den_pool.tile([1, 512], F32, tag="rec", bufs=4,
                                         name="denB")
                    nc.vector.tensor_copy(denA, pcA[64:65, :])
                    nc.vector.tensor_copy(denB, pcB[64:65, :])
                    pbc = ps_main.tile([P, 512], F32, tag="ps")
                    nc.tensor.matmul(pbc[0:64, :], ones1, denA,
                                     start=True, stop=True, tile_position=(0, 0),
                                     skip_group_check=True)
                    nc.tensor.matmul(pbc[64:128, :], ones1, denB,
                                     start=True, stop=True, tile_position=(0, 64),
                                     skip_group_check=True)
                    rbc = work_pool.tile([P, 512], F32, tag="w512")
                    nc.vector.reciprocal(out=rbc, in_=pbc)
                    nc.vector.tensor_mul(ctxT[hp][0:64, qsl], pcA[0:64, :],
                                         rbc[0:64, :])
                    nc.vector.tensor_mul(ctxT[hp][64:128, qsl], pcB[0:64, :],
                                         rbc[64:128, :])

                if phases >= 3:
                    for sti in range(4):
                        st = qr * 4 + sti
                        for nr in range(2):
                            po = ps_main.tile([P, 512], F32, tag="ps")
                            for dc in range(4):
                                nc.tensor.matmul(
                                    po, mmcast(ctxT[dc][:, st * P:(st + 1) * P]),
                                    mmcast(wo[(nr, dc)]),
                                    start=(dc == 0), stop=(dc == 3))
                            ot = work_pool.tile([P, 512], F32, tag="w512")
                            nc.vector.tensor_copy(ot, po)
                            nc.sync.dma_start(
                                out=out_d[st * P:(st + 1) * P,
                                          nr * 512:(nr + 1) * 512],
                                in_=ot)

            if phases < 3:
                for i in range(4):
                    ot = work_pool.tile([P, 512], F32, tag="w512",
                                        name=f"dump{i}")
                    nc.vector.tensor_copy(ot, ctxT[i][:, 0:512].bitcast(F32))
                    nc.sync.dma_start(out=out_d[i * P:(i + 1) * P, 0:512], in_=ot)
    return nc


def _build_program_v3(use_bias: bool, mm_dt: str = "fp32r"):
    """Interleaved emission: projection and output-projection PE work is
    round-robined into the attention instruction stream so the in-order
    PE has filler work while ACT computes exps (keeps HAM warm)."""
    from collections import deque

    import concourse.bass as bass
    import concourse.mybir as mybir
    import concourse.tile as tile
    from concourse.masks import make_identity

    F32 = mybir.dt.float32
    MDT = mybir.dt.float32r if mm_dt == "fp32r" else F32

    def dcast(ap):
        return ap.bitcast(MDT) if MDT is not F32 else ap

    nc = bass.Bass()
    xs = {t: nc.dram_tensor(f"x{t}T", [D, S], F32, kind="ExternalInput")
          for t in "qkv"}
    ws = {t: nc.dram_tensor(f"w{t}T", [D, DG], F32, kind="ExternalInput")
          for t in "qkv"}
    woT = nc.dram_tensor("woT", [DG, D], F32, kind="ExternalInput")
    cos_d = nc.dram_tensor("cos_d", [S, DK], F32, kind="ExternalInput")
    ssg_d = nc.dram_tensor("ssg_d", [S, DK], F32, kind="ExternalInput")
    if use_bias:
        bias_d = nc.dram_tensor("bias_d", [4, DG], F32, kind="ExternalInput")
        ones_d = nc.dram_tensor("ones_d", [1, P], F32, kind="ExternalInput")
    out_d = nc.dram_tensor("out", [S, D], F32, kind="ExternalOutput")

    with tile.TileContext(nc) as tc:
        with tc.tile_pool(name="consts", bufs=1) as consts, \
             tc.tile_pool(name="xT", bufs=8) as xT_pool, \
             tc.tile_pool(name="w", bufs=32) as w_pool, \
             tc.tile_pool(name="nat", bufs=2) as nat_pool, \
             tc.tile_pool(name="kt", bufs=4) as kt_pool, \
             tc.tile_pool(name="qt", bufs=8) as qt_pool, \
             tc.tile_pool(name="vp", bufs=1) as v_pool, \
             tc.tile_pool(name="ctx", bufs=8) as ctx_pool, \
             tc.tile_pool(name="den", bufs=1) as den_pool, \
             tc.tile_pool(name="w512", bufs=4) as work_pool, \
             tc.tile_pool(name="psm", bufs=4, space="PSUM") as ps_main, \
             tc.tile_pool(name="psb", bufs=1, space="PSUM") as ps_bc, \
             tc.tile_pool(name="psc", bufs=3, space="PSUM") as ps_ctx:

            ident = consts.tile([P, P], F32)
            make_identity(nc, ident)
            ones1 = consts.tile([1, 64], F32)
            nc.vector.memset(ones1, 1.0)
            cos_sb = consts.tile([P, ST * DK], F32)
            nc.sync.dma_start(out=cos_sb,
                              in_=cos_d.rearrange("(t p) d -> p t d", p=P))
            ssg_sb = consts.tile([P, ST * DK], F32)
            nc.sync.dma_start(out=ssg_sb,
                              in_=ssg_d.rearrange("(t p) d -> p t d", p=P))
            if use_bias:
                bias_sb = consts.tile([4, DG], F32)
                nc.sync.dma_start(out=bias_sb, in_=bias_d[:, :])
                ones_sb = consts.tile([1, P], F32)
                nc.sync.dma_start(out=ones_sb, in_=ones_d[:, :])

            kT = [kt_pool.tile([P, S], MDT, tag="kt", name=f"kT{i}")
                  for i in range(HPAIRS)]
            v_all = v_pool.tile([P, HG * ST * VSTRIDE], MDT)
            ones_col = consts.tile([P, 1], F32)
            nc.vector.memset(ones_col, 1.0)
            ones_bc = bass.AP(tensor=ones_col.tensor, offset=ones_col.offset,
                              ap=[ones_col.ap[0], [0, HG], [0, ST], [0, 1]])
            v4 = v_all.rearrange("p (h t c) -> p h t c", h=HG, t=ST)
            nc.vector.tensor_copy(v4[:, :, :, DK:DK + 1], ones_bc)

            # all weights resident
            wg = {}
            for ti, t in enumerate("qkv"):
                for cc in range(CC):
                    wt = w_pool.tile([P, DG], MDT, tag="w", name=f"w{t}{cc}")
                    nc.sync.dma_start(out=wt,
                                      in_=dcast(ws[t][cc * P:(cc + 1) * P, :]))
                    wg[(t, cc)] = wt
            wo = {}
            for nr in range(2):
                for dc in range(4):
                    wt = w_pool.tile([P, 512], MDT, tag="w",
                                     name=f"wo{nr}_{dc}")
                    nc.sync.dma_start(
                        out=wt, in_=dcast(woT[dc * P:(dc + 1) * P,
                                               nr * 512:(nr + 1) * 512]))
                    wo[(nr, dc)] = wt

            qts = {}   # (sg, hp) -> [128, 512] MDT
            ctxs = {}  # (qr, hp) -> [128, 512] MDT
            xgs = {}   # (t, sg) -> chunk list
            pending_nat = []

            def flush_transposes():
                while pending_nat:
                    ti, sg, sti, st, nat = pending_nat.pop(0)
                    for hp in range(HPAIRS):
                        pt = ps_main.tile([P, P], F32, tag="ps", name="pt")
                        nc.tensor.transpose(pt, nat[:, hp * P:(hp + 1) * P],
                                            ident)
                        if ti == 0:
                            nc.vector.tensor_copy(
                                qts[(sg, hp)][:, sti * P:(sti + 1) * P], pt)
                        else:
                            nc.vector.tensor_copy(
                                kT[hp][:, st * P:(st + 1) * P], pt)

            def cos_bc(st, half):
                src = cos_sb if half is None else ssg_sb
                width = DK if half is None else 32
                off = st * DK + (0 if half in (None, 0) else 32)
                sl = src[:, off:off + width]
                return bass.AP(tensor=sl.tensor, offset=sl.offset,
                               ap=[sl.ap[0], [0, HG], [1, width]])

            def emit_proj_dma(t, sg):
                xg = [xT_pool.tile([P, 512], MDT, tag="xT",
                                   name=f"x{t}{sg}_{i}") for i in range(CC)]
                for cc in range(CC):
                    nc.sync.dma_start(
                        out=xg[cc],
                        in_=dcast(xs[t][cc * P:(cc + 1) * P,
                                        sg * 512:(sg + 1) * 512]))
                xgs[(t, sg)] = xg

            def emit_proj_unit(ti, t, sg, sti):
                st = sg * 4 + sti
                if sti == 0 and ti == 0:
                    for hp in range(HPAIRS):
                        qts[(sg, hp)] = qt_pool.tile(
                            [P, 512], MDT, tag="qt", name=f"qt{sg}_{hp}")
                xg = xgs[(t, sg)]
                psum = ps_main.tile([P, DG], F32, tag="ps")
                if use_bias:
                    nc.tensor.matmul(psum, ones_sb, bias_sb[ti:ti + 1, :],
                                     start=True, stop=False)
                for cc in range(CC):
                    nc.tensor.matmul(psum, xg[cc][:, sti * P:(sti + 1) * P],
                                     wg[(t, cc)],
                                     start=(cc == 0 and not use_bias),
                                     stop=(cc == CC - 1))
                if ti < 2:
                    flush_transposes()
                    nat = nat_pool.tile([P, DG], F32, tag="nat")
                    tmp = work_pool.tile([P, DG], F32, tag="w512")
                    tmp4 = tmp.rearrange("p (h t d) -> p h t d", h=HG, t=2)
                    ps4 = psum.rearrange("p (h t d) -> p h t d", h=HG, t=2)
                    nc.vector.tensor_mul(
                        nat.rearrange("p (h d) -> p h d", h=HG),
                        psum.rearrange("p (h d) -> p h d", h=HG),
                        cos_bc(st, None))
                    nc.vector.tensor_mul(tmp4[:, :, 0, :], ps4[:, :, 1, :],
                                         cos_bc(st, 0))
                    nc.vector.tensor_mul(tmp4[:, :, 1, :], ps4[:, :, 0, :],
                                         cos_bc(st, 1))
                    nc.vector.tensor_add(nat, nat, tmp)
                    # transposes run one unit later (PE meets them after the
                    # in-order DVE has drained this unit's RoPE chain)
                    pending_nat.append((ti, sg, sti, st, nat))
                else:
                    for h in range(HG):
                        nc.vector.tensor_copy(v4[:, h, st, 0:DK],
                                              psum[:, h * DK:(h + 1) * DK])

            def emit_outproj_unit(qr, sti, nr):
                st = qr * 4 + sti
                po = ps_main.tile([P, 512], F32, tag="ps")
                for dc in range(4):
                    nc.tensor.matmul(po, ctxs[(qr, dc)][:, sti * P:(sti + 1) * P],
                                     wo[(nr, dc)], start=(dc == 0),
                                     stop=(dc == 3))
                ot = work_pool.tile([P, 512], F32, tag="w512")
                nc.scalar.copy(ot, po)
                nc.sync.dma_start(
                    out=out_d[st * P:(st + 1) * P, nr * 512:(nr + 1) * 512],
                    in_=ot)

            is_ge = mybir.AluOpType.is_ge
            Exp = mybir.ActivationFunctionType.Exp

            # prologue: projections for s-group 0, prefetch s-group 1
            for ti, t in enumerate("qkv"):
                emit_proj_dma(t, 0)
                for sti in range(4):
                    emit_proj_unit(ti, t, 0, sti)
            for t in "qkv":
                emit_proj_dma(t, 1)

            from functools import partial
            for qr in range(QR):
                flush_transposes()
                fillers = deque()
                if qr + 1 < QR:
                    for ti, t in enumerate("qkv"):
                        for sti in range(4):
                            fillers.append(
                                partial(emit_proj_unit, ti, t, qr + 1, sti))
                if qr + 2 < QR:
                    # prefetch next-next s-group's x chunks well before use
                    for t in "qkv":
                        fillers.append(partial(emit_proj_dma, t, qr + 2))
                if qr >= 1:
                    for sti in range(4):
                        for nr in range(2):
                            fillers.append(
                                partial(emit_outproj_unit, qr - 1, sti, nr))
                n_slots = 16 * (qr + 1)
                # +HPAIRS: the deferred per-head-pair normalizes are appended
                # while the loop runs; reserve pace slots so they drain
                # interleaved instead of bursting at the qr boundary
                pace = max(1, n_slots // (len(fillers) + HPAIRS))
                cnt = 0
                for hp in range(HPAIRS):
                    hA, hB = 2 * hp, 2 * hp + 1
                    pcA = ps_ctx.tile([VSTRIDE, 512], F32, tag="pc")
                    pcB = ps_ctx.tile([VSTRIDE, 512], F32, tag="pc")
                    n_kc = 4 * (qr + 1)
                    for kc in range(n_kc):
                        psA = ps_main.tile([P, 512], F32, tag="ps")
                        psB = ps_main.tile([P, 512], F32, tag="ps")
                        ksl = slice(kc * P, (kc + 1) * P)
                        nc.tensor.matmul(psA, kT[hp][0:64, ksl],
                                         qts[(qr, hp)][0:64, :],
                                         start=True, stop=True,
                                         tile_position=(0, 0))
                        nc.tensor.matmul(psB, kT[hp][64:128, ksl],
                                         qts[(qr, hp)][64:128, :],
                                         start=True, stop=True,
                                         tile_position=(64, 0))
                        eA = work_pool.tile([P, 512], MDT, tag="w512")
                        eB = work_pool.tile([P, 512], MDT, tag="w512")
                        nc.scalar.activation(out=eA, in_=psA, func=Exp,
                                             scale=0.125)
                        nc.scalar.activation(out=eB, in_=psB, func=Exp,
                                             scale=0.125)
                        j = kc - 4 * qr
                        if j >= 0:
                            for e in (eA, eB):
                                nc.gpsimd.affine_select(
                                    out=e, in_=e, compare_op=is_ge, fill=0.0,
                                    base=-128 * j, channel_multiplier=-1,
                                    pattern=[[1, 512]])
                        nc.tensor.matmul(pcA, v4[:, hA, kc, :], eA,
                                         start=(kc == 0), stop=(kc == n_kc - 1))
                        nc.tensor.matmul(pcB, v4[:, hB, kc, :], eB,
                                         start=(kc == 0), stop=(kc == n_kc - 1))
                        cnt += 1
                        if cnt % pace == 0 and fillers:
                            fillers.popleft()()
                    for hp2 in (hA, hB):
                        pass
                    # Quick-release the ctx PSUM banks: copy out rows
                    # unnormalized, then normalize in SBUF off the PE
                    # critical path (the 3.4us DVE reciprocal otherwise
                    # stalls the next head-pair long enough to re-throttle
                    # the PE clock).
                    denA = den_pool.tile([1, 512], F32, tag="rec", bufs=3,
                                         name="denA")
                    denB = den_pool.tile([1, 512], F32, tag="rec", bufs=3,
                                         name="denB")
                    nc.scalar.copy(denA, pcA[64:65, :])
                    nc.scalar.copy(denB, pcB[64:65, :])
                    ctx = ctx_pool.tile([P, 512], MDT, tag="ctx",
                                        name=f"ctx{qr}_{hp}")
                    ctxs[(qr, hp)] = ctx
                    nc.scalar.copy(ctx[0:64, :], pcA[0:64, :])
                    nc.scalar.copy(ctx[64:128, :], pcB[0:64, :])

                    def emit_normalize(ctx=ctx, denA=denA, denB=denB):
                        # deferred: emitted a few attention slots later so
                        # the PE/DVE never stall at the head-pair boundary
                        pbc = ps_bc.tile([P, 512], F32, tag="pbc",
                                         name="pbc")
                        nc.tensor.matmul(pbc[0:64, :], ones1, denA,
                                         start=True, stop=True,
                                         tile_position=(0, 0),
                                         skip_group_check=True)
                        nc.tensor.matmul(pbc[64:128, :], ones1, denB,
                                         start=True, stop=True,
                                         tile_position=(0, 64),
                                         skip_group_check=True)
                        rbc = work_pool.tile([P, 512], F32, tag="rbc",
                                             bufs=2, name="rbc")
                        nc.vector.reciprocal(out=rbc, in_=pbc)
                        nc.gpsimd.tensor_mul(ctx[0:64, :], ctx[0:64, :],
                                             rbc[0:64, :])
                        nc.gpsimd.tensor_mul(ctx[64:128, :], ctx[64:128, :],
                                             rbc[64:128, :])

                    fillers.append(emit_normalize)
                while fillers:
                    fillers.popleft()()
            for sti in range(4):
                for nr in range(2):
                    emit_outproj_unit(QR - 1, sti, nr)
    return nc


def _build_program_v4(use_bias: bool, mm_dt: str = "fp32r"):
    """v3 plus: (a) diagonal score blocks restricted to their unmasked
    q-range (widths 512/384/256/128 instead of always 512) across the
    scores/exp/select/attn@V chain; (b) the [128,512] DVE reciprocal
    replaced by reciprocal_approx_fast (~5x); (c) attn@V deferred one kc
    behind the score matmuls so the in-order PE doesn't stall on ACT's
    exp latency every iteration."""
    from collections import deque

    import concourse.bass as bass
    import concourse.mybir as mybir
    import concourse.tile as tile
    from concourse.masks import make_identity

    F32 = mybir.dt.float32
    MDT = mybir.dt.float32r if mm_dt == "fp32r" else F32

    def dcast(ap):
        return ap.bitcast(MDT) if MDT is not F32 else ap

    nc = bass.Bass()
    xs = {t: nc.dram_tensor(f"x{t}T", [D, S], F32, kind="ExternalInput")
          for t in "qkv"}
    ws = {t: nc.dram_tensor(f"w{t}T", [D, DG], F32, kind="ExternalInput")
          for t in "qkv"}
    woT = nc.dram_tensor("woT", [DG, D], F32, kind="ExternalInput")
    cos_d = nc.dram_tensor("cos_d", [S, DK], F32, kind="ExternalInput")
    ssg_d = nc.dram_tensor("ssg_d", [S, DK], F32, kind="ExternalInput")
    if use_bias:
        bias_d = nc.dram_tensor("bias_d", [4, DG], F32, kind="ExternalInput")
        ones_d = nc.dram_tensor("ones_d", [1, P], F32, kind="ExternalInput")
    out_d = nc.dram_tensor("out", [S, D], F32, kind="ExternalOutput")

    with tile.TileContext(nc) as tc:
        with tc.tile_pool(name="consts", bufs=1) as consts, \
             tc.tile_pool(name="xT", bufs=8) as xT_pool, \
             tc.tile_pool(name="w", bufs=32) as w_pool, \
             tc.tile_pool(name="nat", bufs=2) as nat_pool, \
             tc.tile_pool(name="kt", bufs=4) as kt_pool, \
             tc.tile_pool(name="qt", bufs=8) as qt_pool, \
             tc.tile_pool(name="vp", bufs=1) as v_pool, \
             tc.tile_pool(name="ctx", bufs=8) as ctx_pool, \
             tc.tile_pool(name="den", bufs=1) as den_pool, \
             tc.tile_pool(name="w512", bufs=4) as work_pool, \
             tc.tile_pool(name="psm", bufs=4, space="PSUM") as ps_main, \
             tc.tile_pool(name="psb", bufs=1, space="PSUM") as ps_bc, \
             tc.tile_pool(name="psc", bufs=3, space="PSUM") as ps_ctx:

            ident = consts.tile([P, P], F32)
            make_identity(nc, ident)
            ones1 = consts.tile([1, 64], F32)
            nc.vector.memset(ones1, 1.0)
            cos_sb = consts.tile([P, ST * DK], F32)
            nc.sync.dma_start(out=cos_sb,
                              in_=cos_d.rearrange("(t p) d -> p t d", p=P))
            ssg_sb = consts.tile([P, ST * DK], F32)
            nc.sync.dma_start(out=ssg_sb,
                              in_=ssg_d.rearrange("(t p) d -> p t d", p=P))
            if use_bias:
                bias_sb = consts.tile([4, DG], F32)
                nc.sync.dma_start(out=bias_sb, in_=bias_d[:, :])
                ones_sb = consts.tile([1, P], F32)
                nc.sync.dma_start(out=ones_sb, in_=ones_d[:, :])

            kT = [kt_pool.tile([P, S], MDT, tag="kt", name=f"kT{i}")
                  for i in range(HPAIRS)]
            v_all = v_pool.tile([P, HG * ST * VSTRIDE], MDT)
            ones_col = consts.tile([P, 1], F32)
            nc.vector.memset(ones_col, 1.0)
            ones_bc = bass.AP(tensor=ones_col.tensor, offset=ones_col.offset,
                              ap=[ones_col.ap[0], [0, HG], [0, ST], [0, 1]])
            v4 = v_all.rearrange("p (h t c) -> p h t c", h=HG, t=ST)
            nc.vector.tensor_copy(v4[:, :, :, DK:DK + 1], ones_bc)

            # all weights resident
            wg = {}
            for ti, t in enumerate("qkv"):
                for cc in range(CC):
                    wt = w_pool.tile([P, DG], MDT, tag="w", name=f"w{t}{cc}")
                    nc.sync.dma_start(out=wt,
                                      in_=dcast(ws[t][cc * P:(cc + 1) * P, :]))
                    wg[(t, cc)] = wt
            wo = {}
            for nr in range(2):
                for dc in range(4):
                    wt = w_pool.tile([P, 512], MDT, tag="w",
                                     name=f"wo{nr}_{dc}")
                    nc.sync.dma_start(
                        out=wt, in_=dcast(woT[dc * P:(dc + 1) * P,
                                               nr * 512:(nr + 1) * 512]))
                    wo[(nr, dc)] = wt

            qts = {}   # (sg, hp) -> [128, 512] MDT
            ctxs = {}  # (qr, hp) -> [128, 512] MDT
            xgs = {}   # (t, sg) -> chunk list
            pending_nat = []

            def flush_transposes():
                while pending_nat:
                    ti, sg, sti, st, nat = pending_nat.pop(0)
                    for hp in range(HPAIRS):
                        pt = ps_main.tile([P, P], F32, tag="ps", name="pt")
                        nc.tensor.transpose(pt, nat[:, hp * P:(hp + 1) * P],
                                            ident)
                        if ti == 0:
                            nc.vector.tensor_copy(
                                qts[(sg, hp)][:, sti * P:(sti + 1) * P], pt)
                        else:
                            nc.vector.tensor_copy(
                                kT[hp][:, st * P:(st + 1) * P], pt)

            def cos_bc(st, half):
                src = cos_sb if half is None else ssg_sb
                width = DK if half is None else 32
                off = st * DK + (0 if half in (None, 0) else 32)
                sl = src[:, off:off + width]
                return bass.AP(tensor=sl.tensor, offset=sl.offset,
                               ap=[sl.ap[0], [0, HG], [1, width]])

            def emit_proj_dma(t, sg):
                xg = [xT_pool.tile([P, 512], MDT, tag="xT",
                                   name=f"x{t}{sg}_{i}") for i in range(CC)]
                for cc in range(CC):
                    nc.sync.dma_start(
                        out=xg[cc],
                        in_=dcast(xs[t][cc * P:(cc + 1) * P,
                                        sg * 512:(sg + 1) * 512]))
                xgs[(t, sg)] = xg

            def emit_proj_unit(ti, t, sg, sti):
                st = sg * 4 + sti
                if sti == 0 and ti == 0:
                    for hp in range(HPAIRS):
                        qts[(sg, hp)] = qt_pool.tile(
                            [P, 512], MDT, tag="qt", name=f"qt{sg}_{hp}")
                xg = xgs[(t, sg)]
                psum = ps_main.tile([P, DG], F32, tag="ps")
                if use_bias:
                    nc.tensor.matmul(psum, ones_sb, bias_sb[ti:ti + 1, :],
                                     start=True, stop=False)
                for cc in range(CC):
                    nc.tensor.matmul(psum, xg[cc][:, sti * P:(sti + 1) * P],
                                     wg[(t, cc)],
                                     start=(cc == 0 and not use_bias),
                                     stop=(cc == CC - 1))
                if ti < 2:
                    flush_transposes()
                    nat = nat_pool.tile([P, DG], F32, tag="nat")
                    tmp = work_pool.tile([P, DG], F32, tag="w512")
                    tmp4 = tmp.rearrange("p (h t d) -> p h t d", h=HG, t=2)
                    ps4 = psum.rearrange("p (h t d) -> p h t d", h=HG, t=2)
                    nc.vector.tensor_mul(
                        nat.rearrange("p (h d) -> p h d", h=HG),
                        psum.rearrange("p (h d) -> p h d", h=HG),
                        cos_bc(st, None))
                    nc.vector.tensor_mul(tmp4[:, :, 0, :], ps4[:, :, 1, :],
                                         cos_bc(st, 0))
                    nc.vector.tensor_mul(tmp4[:, :, 1, :], ps4[:, :, 0, :],
                                         cos_bc(st, 1))
                    nc.vector.tensor_add(nat, nat, tmp)
                    pending_nat.append((ti, sg, sti, st, nat))
                else:
                    for h in range(HG):
                        nc.vector.tensor_copy(v4[:, h, st, 0:DK],
                                              psum[:, h * DK:(h + 1) * DK])

            def emit_outproj_unit(qr, sti, nr):
                st = qr * 4 + sti
                po = ps_main.tile([P, 512], F32, tag="ps")
                for dc in range(4):
                    nc.tensor.matmul(po, ctxs[(qr, dc)][:, sti * P:(sti + 1) * P],
                                     wo[(nr, dc)], start=(dc == 0),
                                     stop=(dc == 3))
                ot = work_pool.tile([P, 512], F32, tag="w512")
                nc.scalar.copy(ot, po)
                nc.sync.dma_start(
                    out=out_d[st * P:(st + 1) * P, nr * 512:(nr + 1) * 512],
                    in_=ot)

            is_ge = mybir.AluOpType.is_ge
            Exp = mybir.ActivationFunctionType.Exp
            Ln = mybir.ActivationFunctionType.Ln

            # prologue: projections for s-group 0, prefetch s-group 1
            for ti, t in enumerate("qkv"):
                emit_proj_dma(t, 0)
                for sti in range(4):
                    emit_proj_unit(ti, t, 0, sti)
            for t in "qkv":
                emit_proj_dma(t, 1)

            from functools import partial
            den_cur = []
            normalizers = deque()
            for qr in range(QR):
                flush_transposes()
                fillers = deque()
                if qr + 1 < QR:
                    for ti, t in enumerate("qkv"):
                        for sti in range(4):
                            fillers.append(
                                partial(emit_proj_unit, ti, t, qr + 1, sti))
                if qr + 2 < QR:
                    for t in "qkv":
                        fillers.append(partial(emit_proj_dma, t, qr + 2))
                if qr >= 1:
                    for sti in range(4):
                        for nr in range(2):
                            fillers.append(
                                partial(emit_outproj_unit, qr - 1, sti, nr))
                n_slots = 16 * (qr + 1)
                pace = max(1, n_slots // (len(fillers) + HPAIRS))
                cnt = 0
                for hp in range(HPAIRS):
                    hA, hB = 2 * hp, 2 * hp + 1
                    pcA = ps_ctx.tile([VSTRIDE, 512], F32, tag="pc")
                    pcB = ps_ctx.tile([VSTRIDE, 512], F32, tag="pc")
                    n_kc = 4 * (qr + 1)

                    def emit_attnv(eA, eB, kc, qoff, width, n_kc=n_kc,
                                   pcA=pcA, pcB=pcB, hA=hA, hB=hB):
                        nc.tensor.matmul(pcA[:, qoff:512],
                                         v4[:, hA, kc, :], eA,
                                         start=(kc == 0),
                                         stop=(kc == n_kc - 1),
                                         skip_group_check=True)
                        nc.tensor.matmul(pcB[:, qoff:512],
                                         v4[:, hB, kc, :], eB,
                                         start=(kc == 0),
                                         stop=(kc == n_kc - 1),
                                         skip_group_check=True)

                    pend = None
                    for kc in range(n_kc):
                        j = kc - 4 * qr
                        qoff = 128 * j if j > 0 else 0
                        width = 512 - qoff
                        psA = ps_main.tile([P, 512], F32, tag="ps")
                        psB = ps_main.tile([P, 512], F32, tag="ps")
                        ksl = slice(kc * P, (kc + 1) * P)
                        qt_r = qts[(qr, hp)]
                        nc.tensor.matmul(psA[:, qoff:512], kT[hp][0:64, ksl],
                                         qt_r[0:64, qoff:512],
                                         start=True, stop=True,
                                         tile_position=(0, 0))
                        nc.tensor.matmul(psB[:, qoff:512], kT[hp][64:128, ksl],
                                         qt_r[64:128, qoff:512],
                                         start=True, stop=True,
                                         tile_position=(64, 0))
                        eA = work_pool.tile([P, width], MDT, tag="w512",
                                            name="eA")
                        eB = work_pool.tile([P, width], MDT, tag="w512",
                                            name="eB")
                        nc.scalar.activation(out=eA, in_=psA[:, qoff:512],
                                             func=Exp, scale=0.125)
                        nc.scalar.activation(out=eB, in_=psB[:, qoff:512],
                                             func=Exp, scale=0.125)
                        if j >= 0:
                            for e in (eA, eB):
                                nc.gpsimd.affine_select(
                                    out=e, in_=e, compare_op=is_ge, fill=0.0,
                                    base=0, channel_multiplier=-1,
                                    pattern=[[1, width]])
                        if pend is not None:
                            emit_attnv(*pend)
                        pend = (eA, eB, kc, qoff, width)
                        cnt += 1
                        if cnt % pace == 0 and fillers:
                            fillers.popleft()()
                    emit_attnv(*pend)
                    denA = den_pool.tile([1, 512], F32, tag="rec", bufs=3,
                                         name="denA")
                    denB = den_pool.tile([1, 512], F32, tag="rec", bufs=3,
                                         name="denB")
                    nc.scalar.copy(denA, pcA[64:65, :])
                    nc.scalar.copy(denB, pcB[64:65, :])
                    ctx = ctx_pool.tile([P, 512], MDT, tag="ctx",
                                        name=f"ctx{qr}_{hp}")
                    ctxs[(qr, hp)] = ctx
                    nc.scalar.copy(ctx[0:64, :], pcA[0:64, :])
                    nc.scalar.copy(ctx[64:128, :], pcB[0:64, :])

                    def emit_normalize(ctx=ctx, denA=denA, denB=denB):
                        # broadcast raw dens to [128,512] via K=1 matmuls,
                        # then 1/x = exp(-ln x) on ACT: Ln/Exp share the
                        # natural_log_exp_and_others table set, so no table
                        # switches interleave with the softmax exps (the DVE
                        # InstReciprocal this replaces cost 3.4us per call).
                        pbc = ps_bc.tile([P, 512], F32, tag="pbc",
                                         name="pbc")
                        nc.tensor.matmul(pbc[0:64, :], ones1, denA,
                                         start=True, stop=True,
                                         tile_position=(0, 0),
                                         skip_group_check=True)
                        nc.tensor.matmul(pbc[64:128, :], ones1, denB,
                                         start=True, stop=True,
                                         tile_position=(0, 64),
                                         skip_group_check=True)
                        rbc = work_pool.tile([P, 512], F32, tag="rbc",
                                             bufs=2, name="rbc")
                        nc.scalar.activation(out=rbc, in_=pbc, func=Ln)
                        nc.scalar.activation(out=rbc, in_=rbc,
                                             func=Exp, scale=-1.0)
                        nc.gpsimd.tensor_mul(ctx[0:64, :], ctx[0:64, :],
                                             rbc[0:64, :])
                        nc.gpsimd.tensor_mul(ctx[64:128, :], ctx[64:128, :],
                                             rbc[64:128, :])

                    fillers.append(emit_normalize)
                while fillers:
                    fillers.popleft()()
            for sti in range(4):
                for nr in range(2):
                    emit_outproj_unit(QR - 1, sti, nr)
    return nc


_PROG_CACHE = {}


def _get_program(use_bias: bool):
    mm_dt = os.environ.get("KERNEL_MM_DT", "fp32r")
    ver = os.environ.get("KERNEL_V", "4")
    key = (use_bias, mm_dt, ver)
    if key not in _PROG_CACHE:
        if ver == "4":
            _PROG_CACHE[key] = _build_program_v4(use_bias, mm_dt=mm_dt)
        elif ver == "3":
            _PROG_CACHE[key] = _build_program_v3(use_bias, mm_dt=mm_dt)
        else:
            _PROG_CACHE[key] = _build_program(use_bias, mm_dt=mm_dt)
    return _PROG_CACHE[key]


def _rope_tables():
    inv = 1.0 / (ROPE_BASE ** (np.arange(0, DK, 2, dtype=np.float32) / DK))
    t = np.arange(S, dtype=np.float32)
    fr = t[:, None] * inv[None, :]                      # [S, 32]
    emb = np.concatenate([fr, fr], axis=-1)             # [S, 64]
    cos = np.cos(emb).astype(np.float32)
    sin = np.sin(emb).astype(np.float32)
    ssg = sin.copy()
    ssg[:, :32] = -sin[:, :32]
    return cos, ssg


def kernel(query, key, value, W_q, b_q, W_k, b_k, W_v, b_v, W_o, b_o):
    _install_patches()
    from concourse.bass_utils import run_bass_kernel_spmd

    query = np.asarray(query, dtype=np.float32)
    key = np.asarray(key, dtype=np.float32)
    value = np.asarray(value, dtype=np.float32)
    W_q, W_k, W_v, W_o = (np.asarray(w, dtype=np.float32)
                          for w in (W_q, W_k, W_v, W_o))
    b_q, b_k, b_v, b_o = (np.asarray(b, dtype=np.float32)
                          for b in (b_q, b_k, b_v, b_o))

    use_bias = bool(np.any(b_q) or np.any(b_k) or np.any(b_v))
    nc = _get_program(use_bias)

    cos, ssg = _rope_tables()

    in_maps = []
    for c in range(N_CORES):
        b, g = divmod(c, 2)
        gs = slice(g * DG, (g + 1) * DG)
        m = {
            "xqT": np.ascontiguousarray(query[b].T),
            "xkT": np.ascontiguousarray(key[b].T),
            "xvT": np.ascontiguousarray(value[b].T),
            "wqT": np.ascontiguousarray(W_q[gs, :].T),
            "wkT": np.ascontiguousarray(W_k[gs, :].T),
            "wvT": np.ascontiguousarray(W_v[gs, :].T),
            "woT": np.ascontiguousarray(W_o[:, gs].T),
            "cos_d": cos,
            "ssg_d": ssg,
        }
        if use_bias:
            m["bias_d"] = np.stack([b_q[gs], b_k[gs], b_v[gs],
                                    np.zeros(DG, np.float32)])
            m["ones_d"] = np.ones((1, P), np.float32)
        in_maps.append(m)

    trace = bool(int(os.environ.get("KERNEL_TRACE", "0")))
    trace_cores = None
    if trace:
        tc_env = os.environ.get("KERNEL_TRACE_CORES", "")
        trace_cores = ([int(x) for x in tc_env.split(",") if x != ""]
                       if tc_env else list(range(N_CORES)))
    try:
        res = run_bass_kernel_spmd(nc, in_maps, core_ids=list(range(N_CORES)),
                                   trace=trace, trace_cores=trace_cores)
    except Exception:
        if not trace:
            raise
        res = run_bass_kernel_spmd(nc, in_maps, core_ids=list(range(N_CORES)),
                                   trace=False)
    kernel._last_results = res

    out = np.empty((B, S, D), np.float32)
    for b in range(B):
        out[b] = res.results[2 * b]["out"] + res.results[2 * b + 1]["out"] + b_o
    return out



# revision 10
# speedup vs baseline: 1.1523x; 1.0748x over previous
"""Multi-head attention (RoPE, causal) Trainium2 kernel, 8-way sharded.

Sharding: core c => batch b = c//2, head-group g = c%2 (8 of 16 heads).
Each core computes Q/K/V projections for its (b, g), RoPE, causal
attention over its 8 heads, and the row-slice of the output projection.
Host sums the two partial output projections per batch and adds b_o.

Per-core dataflow (fp32 storage; matmuls in float32r — single-pass
reduced-precision fp32, ~1.7e-4 per-matmul rel err, 2.7x the throughput
of full fp32's two half-speed passes):
  - projections contract over model dim via PE; x supplied host-transposed
    [D, S] so both operands have the contraction on partitions.
  - Q/K projected to natural [s, d] tiles, RoPE applied with free-dim
    shifted DVE ops, then PE-transposed into QT/KT [d_headpair(128), S].
  - scoresT[k, q] per head via row-paired matmuls (two heads concurrently
    in row-groups 0-1 / 2-3 of the PE array; contraction = dk = 64).
  - softmax without max-subtraction (scores bounded ~|10| for this
    problem); exp on ACT straight out of PSUM; causal masking of diagonal
    blocks via gpsimd affine_select; fully-masked blocks skipped.
  - attn @ V with V' = [V | ones] as stationary (M=65): row 64 accumulates
    the softmax denominator for free. contextT stays unnormalized.
  - normalization: recip(den) broadcast across the 128 head-pair
    partitions with a K=2 indicator matmul, then fused into the PSUM->SBUF
    eviction muls.
  - output projection consumes contextT directly as lhsT (contraction =
    head dims on partitions); per-core result is a [S, D] partial sum.
"""

import json
import os

import numpy as np

# ---------------------------------------------------------------------------
# Workaround: this container's walrus accepts only ONE sync-wait per
# instruction. Hoist every instruction's waits onto single-wait NoOps
# inserted immediately before it (same engine, same program order).
# ---------------------------------------------------------------------------
_PATCHED = False


def _split_multiwait_bir(bir_json: bytes) -> bytes:
    m = json.loads(bir_json)
    ctr = 0
    changed = False
    for f in m.get("functions", []):
        for bl in f.get("blocks", []):
            out = []
            for inst in bl.get("instructions", []):
                si = inst.get("sync_info")
                ow = (si or {}).get("on_wait") or []
                if len(ow) > 1:
                    changed = True
                    for w in ow:
                        ctr += 1
                        out.append({
                            "debug": inst.get("debug", 0),
                            "engine": inst["engine"],
                            "ins": [],
                            "name": f"WSPLIT-{ctr}",
                            "opcode": "NoOp",
                            "outs": [],
                            "sync_info": {"on_update": [], "on_wait": [w]},
                        })
                    si["on_wait"] = []
                out.append(inst)
            if changed:
                bl["instructions"] = out
    if not changed:
        return bir_json
    return json.dumps(m).encode()


def _install_ntff_hook():
    """The agent image's antenv lacks the axon_hooks shim that bass_utils
    imports for trace=True under axon; synthesize it and register the
    ctypes-based NTFF hook from trn_agent_boot (degrades to no-trace if
    anything is missing)."""
    import sys
    import types

    if "antenv.axon_hooks" in sys.modules:
        return
    mod = types.ModuleType("antenv.axon_hooks")
    holder = [None]
    mod.set_axon_ntff_profile_hook = lambda h: holder.__setitem__(0, h)
    mod.get_axon_ntff_profile_hook = lambda: holder[0]
    sys.modules["antenv.axon_hooks"] = mod
    try:
        import antenv
        antenv.axon_hooks = mod
        from trn_agent_boot.trn_boot import _ntff_profile_via_ctypes
        mod.set_axon_ntff_profile_hook(
            _ntff_profile_via_ctypes("/opt/axon/libaxon_pjrt.so"))
    except Exception:
        pass


def _install_patches():
    global _PATCHED
    if _PATCHED:
        return
    import concourse.bass as bass

    orig = bass.Bass.to_json_bytes

    def to_json_bytes_patched(self, *a, **k):
        return _split_multiwait_bir(orig(self, *a, **k))

    bass.Bass.to_json_bytes = to_json_bytes_patched
    _install_ntff_hook()
    _PATCHED = True


# ---------------------------------------------------------------------------
# Problem constants (hardcoded per the harness contract)
# ---------------------------------------------------------------------------
B, S, D = 4, 2048, 1024
H, DK = 16, 64
HG = 8                    # heads per core
DG = HG * DK              # 512: head-group width
N_CORES = 8
ROPE_BASE = 10000.0
P = 128                   # partitions
ST = S // P               # 16 s-tiles
CC = D // P               # 8 contraction chunks for projections
QR = S // 512             # 4 q-ranges of 512
HPAIRS = HG // 2          # 4 head pairs
VSTRIDE = 65              # V columns + ones column


def _build_program(use_bias: bool, phases: int = 3, mm_dt: str = "fp32"):
    import concourse.bass as bass
    import concourse.mybir as mybir
    import concourse.tile as tile
    from concourse.masks import make_identity

    F32 = mybir.dt.float32
    MDT = mybir.dt.float32r if mm_dt == "fp32r" else F32

    def mmcast(ap):
        return ap

    def dcast(ap):
        # DRAM-side view matching MDT-typed SBUF tiles (bit-identical)
        return ap.bitcast(MDT) if MDT is not F32 else ap
    nc = bass.Bass()

    xqT = nc.dram_tensor("xqT", [D, S], F32, kind="ExternalInput")
    xkT = nc.dram_tensor("xkT", [D, S], F32, kind="ExternalInput")
    xvT = nc.dram_tensor("xvT", [D, S], F32, kind="ExternalInput")
    wqT = nc.dram_tensor("wqT", [D, DG], F32, kind="ExternalInput")
    wkT = nc.dram_tensor("wkT", [D, DG], F32, kind="ExternalInput")
    wvT = nc.dram_tensor("wvT", [D, DG], F32, kind="ExternalInput")
    woT = nc.dram_tensor("woT", [DG, D], F32, kind="ExternalInput")
    cos_d = nc.dram_tensor("cos_d", [S, DK], F32, kind="ExternalInput")
    ssg_d = nc.dram_tensor("ssg_d", [S, DK], F32, kind="ExternalInput")
    if use_bias:
        bias_d = nc.dram_tensor("bias_d", [4, DG], F32, kind="ExternalInput")
        ones_d = nc.dram_tensor("ones_d", [1, P], F32, kind="ExternalInput")
    out_d = nc.dram_tensor("out", [S, D], F32, kind="ExternalOutput")

    with tile.TileContext(nc) as tc:
        with tc.tile_pool(name="consts", bufs=1) as consts, \
             tc.tile_pool(name="xT", bufs=8) as xT_pool, \
             tc.tile_pool(name="w", bufs=8) as w_pool, \
             tc.tile_pool(name="nat", bufs=3) as nat_pool, \
             tc.tile_pool(name="qk", bufs=8) as qk_pool, \
             tc.tile_pool(name="vp", bufs=1) as v_pool, \
             tc.tile_pool(name="ctx", bufs=4) as ctx_pool, \
             tc.tile_pool(name="den", bufs=1) as den_pool, \
             tc.tile_pool(name="w512", bufs=6) as work_pool, \
             tc.tile_pool(name="psm", bufs=4, space="PSUM") as ps_main, \
             tc.tile_pool(name="psc", bufs=4, space="PSUM") as ps_ctx:

            ident = consts.tile([P, P], F32)
            make_identity(nc, ident)
            ones1 = consts.tile([1, 64], F32)
            nc.vector.memset(ones1, 1.0)
            # cos/ssign: [S, 64] -> [128, 16*64] (s = st*128 + p)
            cos_sb = consts.tile([P, ST * DK], F32)
            nc.sync.dma_start(out=cos_sb,
                              in_=cos_d.rearrange("(t p) d -> p t d", p=P))
            ssg_sb = consts.tile([P, ST * DK], F32)
            nc.sync.dma_start(out=ssg_sb,
                              in_=ssg_d.rearrange("(t p) d -> p t d", p=P))
            if use_bias:
                bias_sb = consts.tile([4, DG], F32)
                nc.sync.dma_start(out=bias_sb, in_=bias_d[:, :])
                ones_sb = consts.tile([1, P], F32)
                nc.sync.dma_start(out=ones_sb, in_=ones_d[:, :])

            # persistent activations
            qT = [qk_pool.tile([P, S], MDT, tag="qk", name=f"qT{i}") for i in range(HPAIRS)]
            kT = [qk_pool.tile([P, S], MDT, tag="qk", name=f"kT{i}") for i in range(HPAIRS)]
            v_all = v_pool.tile([P, HG * ST * VSTRIDE], MDT)
            # ones columns of V' (single strided broadcast copy)
            ones_col = consts.tile([P, 1], F32)
            nc.vector.memset(ones_col, 1.0)
            ones_bc = bass.AP(tensor=ones_col.tensor, offset=ones_col.offset,
                              ap=[ones_col.ap[0], [0, HG], [0, ST], [0, 1]])
            nc.vector.tensor_copy(
                v_all.rearrange("p (h t c) -> p h t c", h=HG, t=ST)[:, :, :, DK:DK + 1],
                ones_bc)
            ctxT = [ctx_pool.tile([P, S], MDT, tag="ctx", name=f"ctxT{i}") for i in range(HPAIRS)]

            # ---------------- projections + RoPE + transposes --------------
            def cos_bc(st, half):
                # cos/ssign slice [128, 32] broadcast over 8 heads
                src = cos_sb if half is None else ssg_sb
                width = DK if half is None else 32
                off = st * DK + (0 if half in (None, 0) else 32)
                sl = src[:, off:off + width]
                return bass.AP(tensor=sl.tensor, offset=sl.offset,
                               ap=[sl.ap[0], [0, HG], [1, width]])

            for t_i, (x_t, w_t) in enumerate(((xqT, wqT), (xkT, wkT), (xvT, wvT))):
                for sg in range(QR):           # groups of 4 s-tiles
                    xg = [xT_pool.tile([P, 512], MDT, tag="xT", name=f"xg{i}") for i in range(CC)]
                    for cc in range(CC):
                        nc.sync.dma_start(
                            out=xg[cc],
                            in_=dcast(x_t[cc * P:(cc + 1) * P,
                                          sg * 512:(sg + 1) * 512]))
                    if sg == 0:
                        wg = [w_pool.tile([P, DG], MDT, tag="w", name=f"wg{i}") for i in range(CC)]
                        for cc in range(CC):
                            nc.sync.dma_start(
                                out=wg[cc],
                                in_=dcast(w_t[cc * P:(cc + 1) * P, :]))
                    for sti in range(4):
                        st = sg * 4 + sti
                        psum = ps_main.tile([P, DG], F32, tag="ps")
                        if use_bias:
                            nc.tensor.matmul(psum, ones_sb,
                                             bias_sb[t_i:t_i + 1, :],
                                             start=True, stop=False)
                        for cc in range(CC):
                            nc.tensor.matmul(
                                psum, mmcast(xg[cc][:, sti * P:(sti + 1) * P]),
                                mmcast(wg[cc]),
                                start=(cc == 0 and not use_bias),
                                stop=(cc == CC - 1))
                        if t_i < 2:
                            # RoPE: nat = psum*cos ; nat += shift(psum)*ssign
                            nat = nat_pool.tile([P, DG], F32, tag="nat")
                            tmp = work_pool.tile([P, DG], F32, tag="w512")
                            nat4 = nat.rearrange("p (h t d) -> p h t d", h=HG, t=2)
                            tmp4 = tmp.rearrange("p (h t d) -> p h t d", h=HG, t=2)
                            ps4 = psum.rearrange("p (h t d) -> p h t d", h=HG, t=2)
                            nc.vector.tensor_mul(
                                nat.rearrange("p (h d) -> p h d", h=HG),
                                psum.rearrange("p (h d) -> p h d", h=HG),
                                cos_bc(st, None))
                            nc.vector.tensor_mul(tmp4[:, :, 0, :], ps4[:, :, 1, :],
                                                 cos_bc(st, 0))
                            nc.vector.tensor_mul(tmp4[:, :, 1, :], ps4[:, :, 0, :],
                                                 cos_bc(st, 1))
                            nc.vector.tensor_add(nat, nat, tmp)
                            dest = qT if t_i == 0 else kT
                            for hp in range(HPAIRS):
                                pt = ps_ctx.tile([P, P], F32, tag="pc")
                                nc.tensor.transpose(
                                    pt, nat[:, hp * P:(hp + 1) * P], ident)
                                nc.vector.tensor_copy(
                                    dest[hp][:, st * P:(st + 1) * P], pt)
                        else:
                            v4 = v_all.rearrange("p (h t c) -> p h t c",
                                                 h=HG, t=ST)
                            for h in range(HG):
                                nc.vector.tensor_copy(
                                    v4[:, h, st, 0:DK],
                                    psum[:, h * DK:(h + 1) * DK])

            if phases < 2:
                for i in range(4):
                    ot = work_pool.tile([P, 512], F32, tag="w512",
                                        name=f"dump{i}")
                    nc.vector.tensor_copy(ot, qT[i][:, 0:512].bitcast(F32))
                    nc.sync.dma_start(out=out_d[i * P:(i + 1) * P, 0:512], in_=ot)
                return nc
            # ------------- attention + inlined output projection ------------
            # qr-outer so each q-range's output projection follows right
            # after its attention, giving PE dense filler work while ACT
            # works through the exps (keeps HAM warm).
            is_ge = mybir.AluOpType.is_ge
            Exp = mybir.ActivationFunctionType.Exp
            wo = {}
            if phases >= 3:
                for nr in range(2):
                    for dc in range(4):
                        wt = w_pool.tile([P, 512], MDT, tag="w",
                                         name=f"wo{nr}_{dc}")
                        nc.sync.dma_start(
                            out=wt,
                            in_=dcast(woT[dc * P:(dc + 1) * P,
                                          nr * 512:(nr + 1) * 512]))
                        wo[(nr, dc)] = wt
            for qr in range(QR):
                for hp in range(HPAIRS):
                    hA, hB = 2 * hp, 2 * hp + 1
                    pcA = ps_ctx.tile([VSTRIDE, 512], F32, tag="pc")
                    pcB = ps_ctx.tile([VSTRIDE, 512], F32, tag="pc")
                    n_kc = 4 * (qr + 1)
                    for kc in range(n_kc):
                        psA = ps_main.tile([P, 512], F32, tag="ps")
                        psB = ps_main.tile([P, 512], F32, tag="ps")
                        qsl = slice(qr * 512, (qr + 1) * 512)
                        ksl = slice(kc * P, (kc + 1) * P)
                        nc.tensor.matmul(psA, mmcast(kT[hp][0:64, ksl]),
                                         mmcast(qT[hp][0:64, qsl]),
                                         start=True, stop=True, tile_position=(0, 0))
                        nc.tensor.matmul(psB, mmcast(kT[hp][64:128, ksl]),
                                         mmcast(qT[hp][64:128, qsl]),
                                         start=True, stop=True, tile_position=(64, 0))
                        eA = work_pool.tile([P, 512], MDT, tag="w512")
                        eB = work_pool.tile([P, 512], MDT, tag="w512")
                        nc.scalar.activation(out=eA, in_=psA, func=Exp, scale=0.125)
                        nc.scalar.activation(out=eB, in_=psB, func=Exp, scale=0.125)
                        j = kc - 4 * qr
                        if j >= 0:  # diagonal block: keep qq - kk - 128*j >= 0
                            for e in (eA, eB):
                                nc.gpsimd.affine_select(
                                    out=e, in_=e, compare_op=is_ge, fill=0.0,
                                    base=-128 * j, channel_multiplier=-1,
                                    pattern=[[1, 512]])
                        v4 = v_all.rearrange("p (h t c) -> p h t c", h=HG, t=ST)
                        nc.tensor.matmul(pcA, mmcast(v4[:, hA, kc, :]), mmcast(eA),
                                         start=(kc == 0), stop=(kc == n_kc - 1))
                        nc.tensor.matmul(pcB, mmcast(v4[:, hB, kc, :]), mmcast(eB),
                                         start=(kc == 0), stop=(kc == n_kc - 1))
                    qsl = slice(qr * 512, (qr + 1) * 512)
                    denA = den_pool.tile([1, 512], F32, tag="rec", bufs=4,
                                         name="denA")
                    denB = den_pool.tile([1, 512], F32, tag="rec", bufs=4,
                                         name="denB")
                    nc.vector.tensor_copy(denA, pcA[64:65, :])
                    nc.vector.tensor_copy(denB, pcB[64:65, :])
                    pbc = ps_main.tile([P, 512], F32, tag="ps")
                    nc.tensor.matmul(pbc[0:64, :], ones1, denA,
                                     start=True, stop=True, tile_position=(0, 0),
                                     skip_group_check=True)
                    nc.tensor.matmul(pbc[64:128, :], ones1, denB,
                                     start=True, stop=True, tile_position=(0, 64),
                                     skip_group_check=True)
                    rbc = work_pool.tile([P, 512], F32, tag="w512")
                    nc.vector.reciprocal(out=rbc, in_=pbc)
                    nc.vector.tensor_mul(ctxT[hp][0:64, qsl], pcA[0:64, :],
                                         rbc[0:64, :])
                    nc.vector.tensor_mul(ctxT[hp][64:128, qsl], pcB[0:64, :],
                                         rbc[64:128, :])

                if phases >= 3:
                    for sti in range(4):
                        st = qr * 4 + sti
                        for nr in range(2):
                            po = ps_main.tile([P, 512], F32, tag="ps")
                            for dc in range(4):
                                nc.tensor.matmul(
                                    po, mmcast(ctxT[dc][:, st * P:(st + 1) * P]),
                                    mmcast(wo[(nr, dc)]),
                                    start=(dc == 0), stop=(dc == 3))
                            ot = work_pool.tile([P, 512], F32, tag="w512")
                            nc.vector.tensor_copy(ot, po)
                            nc.sync.dma_start(
                                out=out_d[st * P:(st + 1) * P,
                                          nr * 512:(nr + 1) * 512],
                                in_=ot)

            if phases < 3:
                for i in range(4):
                    ot = work_pool.tile([P, 512], F32, tag="w512",
                                        name=f"dump{i}")
                    nc.vector.tensor_copy(ot, ctxT[i][:, 0:512].bitcast(F32))
                    nc.sync.dma_start(out=out_d[i * P:(i + 1) * P, 0:512], in_=ot)
    return nc


def _build_program_v3(use_bias: bool, mm_dt: str = "fp32r"):
    """Interleaved emission: projection and output-projection PE work is
    round-robined into the attention instruction stream so the in-order
    PE has filler work while ACT computes exps (keeps HAM warm)."""
    from collections import deque

    import concourse.bass as bass
    import concourse.mybir as mybir
    import concourse.tile as tile
    from concourse.masks import make_identity

    F32 = mybir.dt.float32
    MDT = mybir.dt.float32r if mm_dt == "fp32r" else F32

    def dcast(ap):
        return ap.bitcast(MDT) if MDT is not F32 else ap

    nc = bass.Bass()
    xs = {t: nc.dram_tensor(f"x{t}T", [D, S], F32, kind="ExternalInput")
          for t in "qkv"}
    ws = {t: nc.dram_tensor(f"w{t}T", [D, DG], F32, kind="ExternalInput")
          for t in "qkv"}
    woT = nc.dram_tensor("woT", [DG, D], F32, kind="ExternalInput")
    cos_d = nc.dram_tensor("cos_d", [S, DK], F32, kind="ExternalInput")
    ssg_d = nc.dram_tensor("ssg_d", [S, DK], F32, kind="ExternalInput")
    if use_bias:
        bias_d = nc.dram_tensor("bias_d", [4, DG], F32, kind="ExternalInput")
        ones_d = nc.dram_tensor("ones_d", [1, P], F32, kind="ExternalInput")
    out_d = nc.dram_tensor("out", [S, D], F32, kind="ExternalOutput")

    with tile.TileContext(nc) as tc:
        with tc.tile_pool(name="consts", bufs=1) as consts, \
             tc.tile_pool(name="xT", bufs=8) as xT_pool, \
             tc.tile_pool(name="w", bufs=32) as w_pool, \
             tc.tile_pool(name="nat", bufs=2) as nat_pool, \
             tc.tile_pool(name="kt", bufs=4) as kt_pool, \
             tc.tile_pool(name="qt", bufs=8) as qt_pool, \
             tc.tile_pool(name="vp", bufs=1) as v_pool, \
             tc.tile_pool(name="ctx", bufs=8) as ctx_pool, \
             tc.tile_pool(name="den", bufs=1) as den_pool, \
             tc.tile_pool(name="w512", bufs=4) as work_pool, \
             tc.tile_pool(name="psm", bufs=4, space="PSUM") as ps_main, \
             tc.tile_pool(name="psb", bufs=1, space="PSUM") as ps_bc, \
             tc.tile_pool(name="psc", bufs=3, space="PSUM") as ps_ctx:

            ident = consts.tile([P, P], F32)
            make_identity(nc, ident)
            ones1 = consts.tile([1, 64], F32)
            nc.vector.memset(ones1, 1.0)
            cos_sb = consts.tile([P, ST * DK], F32)
            nc.sync.dma_start(out=cos_sb,
                              in_=cos_d.rearrange("(t p) d -> p t d", p=P))
            ssg_sb = consts.tile([P, ST * DK], F32)
            nc.sync.dma_start(out=ssg_sb,
                              in_=ssg_d.rearrange("(t p) d -> p t d", p=P))
            if use_bias:
                bias_sb = consts.tile([4, DG], F32)
                nc.sync.dma_start(out=bias_sb, in_=bias_d[:, :])
                ones_sb = consts.tile([1, P], F32)
                nc.sync.dma_start(out=ones_sb, in_=ones_d[:, :])

            kT = [kt_pool.tile([P, S], MDT, tag="kt", name=f"kT{i}")
                  for i in range(HPAIRS)]
            v_all = v_pool.tile([P, HG * ST * VSTRIDE], MDT)
            ones_col = consts.tile([P, 1], F32)
            nc.vector.memset(ones_col, 1.0)
            ones_bc = bass.AP(tensor=ones_col.tensor, offset=ones_col.offset,
                              ap=[ones_col.ap[0], [0, HG], [0, ST], [0, 1]])
            v4 = v_all.rearrange("p (h t c) -> p h t c", h=HG, t=ST)
            nc.vector.tensor_copy(v4[:, :, :, DK:DK + 1], ones_bc)

            # all weights resident
            wg = {}
            for ti, t in enumerate("qkv"):
                for cc in range(CC):
                    wt = w_pool.tile([P, DG], MDT, tag="w", name=f"w{t}{cc}")
                    nc.sync.dma_start(out=wt,
                                      in_=dcast(ws[t][cc * P:(cc + 1) * P, :]))
                    wg[(t, cc)] = wt
            wo = {}
            for nr in range(2):
                for dc in range(4):
                    wt = w_pool.tile([P, 512], MDT, tag="w",
                                     name=f"wo{nr}_{dc}")
                    nc.sync.dma_start(
                        out=wt, in_=dcast(woT[dc * P:(dc + 1) * P,
                                               nr * 512:(nr + 1) * 512]))
                    wo[(nr, dc)] = wt

            qts = {}   # (sg, hp) -> [128, 512] MDT
            ctxs = {}  # (qr, hp) -> [128, 512] MDT
            xgs = {}   # (t, sg) -> chunk list
            pending_nat = []

            def flush_transposes():
                while pending_nat:
                    ti, sg, sti, st, nat = pending_nat.pop(0)
                    for hp in range(HPAIRS):
                        pt = ps_main.tile([P, P], F32, tag="ps", name="pt")
                        nc.tensor.transpose(pt, nat[:, hp * P:(hp + 1) * P],
                                            ident)
                        if ti == 0:
                            nc.vector.tensor_copy(
                                qts[(sg, hp)][:, sti * P:(sti + 1) * P], pt)
                        else:
                            nc.vector.tensor_copy(
                                kT[hp][:, st * P:(st + 1) * P], pt)

            def cos_bc(st, half):
                src = cos_sb if half is None else ssg_sb
                width = DK if half is None else 32
                off = st * DK + (0 if half in (None, 0) else 32)
                sl = src[:, off:off + width]
                return bass.AP(tensor=sl.tensor, offset=sl.offset,
                               ap=[sl.ap[0], [0, HG], [1, width]])

            def emit_proj_dma(t, sg):
                xg = [xT_pool.tile([P, 512], MDT, tag="xT",
                                   name=f"x{t}{sg}_{i}") for i in range(CC)]
                for cc in range(CC):
                    nc.sync.dma_start(
                        out=xg[cc],
                        in_=dcast(xs[t][cc * P:(cc + 1) * P,
                                        sg * 512:(sg + 1) * 512]))
                xgs[(t, sg)] = xg

            def emit_proj_unit(ti, t, sg, sti):
                st = sg * 4 + sti
                if sti == 0 and ti == 0:
                    for hp in range(HPAIRS):
                        qts[(sg, hp)] = qt_pool.tile(
                            [P, 512], MDT, tag="qt", name=f"qt{sg}_{hp}")
                xg = xgs[(t, sg)]
                psum = ps_main.tile([P, DG], F32, tag="ps")
                if use_bias:
                    nc.tensor.matmul(psum, ones_sb, bias_sb[ti:ti + 1, :],
                                     start=True, stop=False)
                for cc in range(CC):
                    nc.tensor.matmul(psum, xg[cc][:, sti * P:(sti + 1) * P],
                                     wg[(t, cc)],
                                     start=(cc == 0 and not use_bias),
                                     stop=(cc == CC - 1))
                if ti < 2:
                    flush_transposes()
                    nat = nat_pool.tile([P, DG], F32, tag="nat")
                    tmp = work_pool.tile([P, DG], F32, tag="w512")
                    tmp4 = tmp.rearrange("p (h t d) -> p h t d", h=HG, t=2)
                    ps4 = psum.rearrange("p (h t d) -> p h t d", h=HG, t=2)
                    nc.vector.tensor_mul(
                        nat.rearrange("p (h d) -> p h d", h=HG),
                        psum.rearrange("p (h d) -> p h d", h=HG),
                        cos_bc(st, None))
                    nc.vector.tensor_mul(tmp4[:, :, 0, :], ps4[:, :, 1, :],
                                         cos_bc(st, 0))
                    nc.vector.tensor_mul(tmp4[:, :, 1, :], ps4[:, :, 0, :],
                                         cos_bc(st, 1))
                    nc.vector.tensor_add(nat, nat, tmp)
                    # transposes run one unit later (PE meets them after the
                    # in-order DVE has drained this unit's RoPE chain)
                    pending_nat.append((ti, sg, sti, st, nat))
                else:
                    for h in range(HG):
                        nc.vector.tensor_copy(v4[:, h, st, 0:DK],
                                              psum[:, h * DK:(h + 1) * DK])

            def emit_outproj_unit(qr, sti, nr):
                st = qr * 4 + sti
                po = ps_main.tile([P, 512], F32, tag="ps")
                for dc in range(4):
                    nc.tensor.matmul(po, ctxs[(qr, dc)][:, sti * P:(sti + 1) * P],
                                     wo[(nr, dc)], start=(dc == 0),
                                     stop=(dc == 3))
                ot = work_pool.tile([P, 512], F32, tag="w512")
                nc.scalar.copy(ot, po)
                nc.sync.dma_start(
                    out=out_d[st * P:(st + 1) * P, nr * 512:(nr + 1) * 512],
                    in_=ot)

            is_ge = mybir.AluOpType.is_ge
            Exp = mybir.ActivationFunctionType.Exp

            # prologue: projections for s-group 0, prefetch s-group 1
            for ti, t in enumerate("qkv"):
                emit_proj_dma(t, 0)
                for sti in range(4):
                    emit_proj_unit(ti, t, 0, sti)
            for t in "qkv":
                emit_proj_dma(t, 1)

            from functools import partial
            for qr in range(QR):
                flush_transposes()
                fillers = deque()
                if qr + 1 < QR:
                    for ti, t in enumerate("qkv"):
                        for sti in range(4):
                            fillers.append(
                                partial(emit_proj_unit, ti, t, qr + 1, sti))
                if qr + 2 < QR:
                    # prefetch next-next s-group's x chunks well before use
                    for t in "qkv":
                        fillers.append(partial(emit_proj_dma, t, qr + 2))
                if qr >= 1:
                    for sti in range(4):
                        for nr in range(2):
                            fillers.append(
                                partial(emit_outproj_unit, qr - 1, sti, nr))
                n_slots = 16 * (qr + 1)
                # +HPAIRS: the deferred per-head-pair normalizes are appended
                # while the loop runs; reserve pace slots so they drain
                # interleaved instead of bursting at the qr boundary
                pace = max(1, n_slots // (len(fillers) + HPAIRS))
                cnt = 0
                for hp in range(HPAIRS):
                    hA, hB = 2 * hp, 2 * hp + 1
                    pcA = ps_ctx.tile([VSTRIDE, 512], F32, tag="pc")
                    pcB = ps_ctx.tile([VSTRIDE, 512], F32, tag="pc")
                    n_kc = 4 * (qr + 1)
                    for kc in range(n_kc):
                        psA = ps_main.tile([P, 512], F32, tag="ps")
                        psB = ps_main.tile([P, 512], F32, tag="ps")
                        ksl = slice(kc * P, (kc + 1) * P)
                        nc.tensor.matmul(psA, kT[hp][0:64, ksl],
                                         qts[(qr, hp)][0:64, :],
                                         start=True, stop=True,
                                         tile_position=(0, 0))
                        nc.tensor.matmul(psB, kT[hp][64:128, ksl],
                                         qts[(qr, hp)][64:128, :],
                                         start=True, stop=True,
                                         tile_position=(64, 0))
                        eA = work_pool.tile([P, 512], MDT, tag="w512")
                        eB = work_pool.tile([P, 512], MDT, tag="w512")
                        nc.scalar.activation(out=eA, in_=psA, func=Exp,
                                             scale=0.125)
                        nc.scalar.activation(out=eB, in_=psB, func=Exp,
                                             scale=0.125)
                        j = kc - 4 * qr
                        if j >= 0:
                            for e in (eA, eB):
                                nc.gpsimd.affine_select(
                                    out=e, in_=e, compare_op=is_ge, fill=0.0,
                                    base=-128 * j, channel_multiplier=-1,
                                    pattern=[[1, 512]])
                        nc.tensor.matmul(pcA, v4[:, hA, kc, :], eA,
                                         start=(kc == 0), stop=(kc == n_kc - 1))
                        nc.tensor.matmul(pcB, v4[:, hB, kc, :], eB,
                                         start=(kc == 0), stop=(kc == n_kc - 1))
                        cnt += 1
                        if cnt % pace == 0 and fillers:
                            fillers.popleft()()
                    for hp2 in (hA, hB):
                        pass
                    # Quick-release the ctx PSUM banks: copy out rows
                    # unnormalized, then normalize in SBUF off the PE
                    # critical path (the 3.4us DVE reciprocal otherwise
                    # stalls the next head-pair long enough to re-throttle
                    # the PE clock).
                    denA = den_pool.tile([1, 512], F32, tag="rec", bufs=3,
                                         name="denA")
                    denB = den_pool.tile([1, 512], F32, tag="rec", bufs=3,
                                         name="denB")
                    nc.scalar.copy(denA, pcA[64:65, :])
                    nc.scalar.copy(denB, pcB[64:65, :])
                    ctx = ctx_pool.tile([P, 512], MDT, tag="ctx",
                                        name=f"ctx{qr}_{hp}")
                    ctxs[(qr, hp)] = ctx
                    nc.scalar.copy(ctx[0:64, :], pcA[0:64, :])
                    nc.scalar.copy(ctx[64:128, :], pcB[0:64, :])

                    def emit_normalize(ctx=ctx, denA=denA, denB=denB):
                        # deferred: emitted a few attention slots later so
                        # the PE/DVE never stall at the head-pair boundary
                        pbc = ps_bc.tile([P, 512], F32, tag="pbc",
                                         name="pbc")
                        nc.tensor.matmul(pbc[0:64, :], ones1, denA,
                                         start=True, stop=True,
                                         tile_position=(0, 0),
                                         skip_group_check=True)
                        nc.tensor.matmul(pbc[64:128, :], ones1, denB,
                                         start=True, stop=True,
                                         tile_position=(0, 64),
                                         skip_group_check=True)
                        rbc = work_pool.tile([P, 512], F32, tag="rbc",
                                             bufs=2, name="rbc")
                        nc.vector.reciprocal(out=rbc, in_=pbc)
                        nc.gpsimd.tensor_mul(ctx[0:64, :], ctx[0:64, :],
                                             rbc[0:64, :])
                        nc.gpsimd.tensor_mul(ctx[64:128, :], ctx[64:128, :],
                                             rbc[64:128, :])

                    fillers.append(emit_normalize)
                while fillers:
                    fillers.popleft()()
            for sti in range(4):
                for nr in range(2):
                    emit_outproj_unit(QR - 1, sti, nr)
    return nc


def _build_program_v4(use_bias: bool, mm_dt: str = "fp32r"):
    """v3 plus: (a) diagonal score blocks restricted to their unmasked
    q-range (widths 512/384/256/128 instead of always 512) across the
    scores/exp/select/attn@V chain; (b) the [128,512] DVE reciprocal
    replaced by reciprocal_approx_fast (~5x); (c) attn@V deferred one kc
    behind the score matmuls so the in-order PE doesn't stall on ACT's
    exp latency every iteration."""
    from collections import deque

    import concourse.bass as bass
    import concourse.mybir as mybir
    import concourse.tile as tile
    from concourse.masks import make_identity

    F32 = mybir.dt.float32
    MDT = mybir.dt.float32r if mm_dt == "fp32r" else F32

    def dcast(ap):
        return ap.bitcast(MDT) if MDT is not F32 else ap

    nc = bass.Bass()
    xs = {t: nc.dram_tensor(f"x{t}T", [D, S], F32, kind="ExternalInput")
          for t in "qkv"}
    ws = {t: nc.dram_tensor(f"w{t}T", [D, DG], F32, kind="ExternalInput")
          for t in "qkv"}
    woT = nc.dram_tensor("woT", [DG, D], F32, kind="ExternalInput")
    cos_d = nc.dram_tensor("cos_d", [S, DK], F32, kind="ExternalInput")
    ssg_d = nc.dram_tensor("ssg_d", [S, DK], F32, kind="ExternalInput")
    if use_bias:
        bias_d = nc.dram_tensor("bias_d", [4, DG], F32, kind="ExternalInput")
        ones_d = nc.dram_tensor("ones_d", [1, P], F32, kind="ExternalInput")
    out_d = nc.dram_tensor("out", [S, D], F32, kind="ExternalOutput")

    with tile.TileContext(nc) as tc:
        with tc.tile_pool(name="consts", bufs=1) as consts, \
             tc.tile_pool(name="xT", bufs=8) as xT_pool, \
             tc.tile_pool(name="w", bufs=32) as w_pool, \
             tc.tile_pool(name="nat", bufs=2) as nat_pool, \
             tc.tile_pool(name="kt", bufs=4) as kt_pool, \
             tc.tile_pool(name="qt", bufs=8) as qt_pool, \
             tc.tile_pool(name="vp", bufs=1) as v_pool, \
             tc.tile_pool(name="ctx", bufs=8) as ctx_pool, \
             tc.tile_pool(name="den", bufs=1) as den_pool, \
             tc.tile_pool(name="w512", bufs=4) as work_pool, \
             tc.tile_pool(name="psm", bufs=4, space="PSUM") as ps_main, \
             tc.tile_pool(name="psb", bufs=1, space="PSUM") as ps_bc, \
             tc.tile_pool(name="psc", bufs=3, space="PSUM") as ps_ctx:

            ident = consts.tile([P, P], F32)
            make_identity(nc, ident)
            ones1 = consts.tile([1, 64], F32)
            nc.vector.memset(ones1, 1.0)
            cos_sb = consts.tile([P, ST * DK], F32)
            nc.sync.dma_start(out=cos_sb,
                              in_=cos_d.rearrange("(t p) d -> p t d", p=P))
            ssg_sb = consts.tile([P, ST * DK], F32)
            nc.sync.dma_start(out=ssg_sb,
                              in_=ssg_d.rearrange("(t p) d -> p t d", p=P))
            if use_bias:
                bias_sb = consts.tile([4, DG], F32)
                nc.sync.dma_start(out=bias_sb, in_=bias_d[:, :])
                ones_sb = consts.tile([1, P], F32)
                nc.sync.dma_start(out=ones_sb, in_=ones_d[:, :])

            kT = [kt_pool.tile([P, S], MDT, tag="kt", name=f"kT{i}")
                  for i in range(HPAIRS)]
            v_all = v_pool.tile([P, HG * ST * VSTRIDE], MDT)
            ones_col = consts.tile([P, 1], F32)
            nc.vector.memset(ones_col, 1.0)
            ones_bc = bass.AP(tensor=ones_col.tensor, offset=ones_col.offset,
                              ap=[ones_col.ap[0], [0, HG], [0, ST], [0, 1]])
            v4 = v_all.rearrange("p (h t c) -> p h t c", h=HG, t=ST)
            nc.vector.tensor_copy(v4[:, :, :, DK:DK + 1], ones_bc)

            # all weights resident
            wg = {}
            for ti, t in enumerate("qkv"):
                for cc in range(CC):
                    wt = w_pool.tile([P, DG], MDT, tag="w", name=f"w{t}{cc}")
                    nc.sync.dma_start(out=wt,
                                      in_=dcast(ws[t][cc * P:(cc + 1) * P, :]))
                    wg[(t, cc)] = wt
            wo = {}
            for nr in range(2):
                for dc in range(4):
                    wt = w_pool.tile([P, 512], MDT, tag="w",
                                     name=f"wo{nr}_{dc}")
                    nc.sync.dma_start(
                        out=wt, in_=dcast(woT[dc * P:(dc + 1) * P,
                                               nr * 512:(nr + 1) * 512]))
                    wo[(nr, dc)] = wt

            qts = {}   # (sg, hp) -> [128, 512] MDT
            ctxs = {}  # (qr, hp) -> [128, 512] MDT
            xgs = {}   # (t, sg) -> chunk list
            pending_nat = []

            def flush_transposes():
                while pending_nat:
                    ti, sg, sti, st, nat = pending_nat.pop(0)
                    for hp in range(HPAIRS):
                        pt = ps_main.tile([P, P], F32, tag="ps", name="pt")
                        nc.tensor.transpose(pt, nat[:, hp * P:(hp + 1) * P],
                                            ident)
                        if ti == 0:
                            nc.vector.tensor_copy(
                                qts[(sg, hp)][:, sti * P:(sti + 1) * P], pt)
                        else:
                            nc.vector.tensor_copy(
                                kT[hp][:, st * P:(st + 1) * P], pt)

            def cos_bc(st, half):
                src = cos_sb if half is None else ssg_sb
                width = DK if half is None else 32
                off = st * DK + (0 if half in (None, 0) else 32)
                sl = src[:, off:off + width]
                return bass.AP(tensor=sl.tensor, offset=sl.offset,
                               ap=[sl.ap[0], [0, HG], [1, width]])

            def emit_proj_dma(t, sg):
                xg = [xT_pool.tile([P, 512], MDT, tag="xT",
                                   name=f"x{t}{sg}_{i}") for i in range(CC)]
                for cc in range(CC):
                    nc.sync.dma_start(
                        out=xg[cc],
                        in_=dcast(xs[t][cc * P:(cc + 1) * P,
                                        sg * 512:(sg + 1) * 512]))
                xgs[(t, sg)] = xg

            def emit_proj_unit(ti, t, sg, sti):
                st = sg * 4 + sti
                if sti == 0 and ti == 0:
                    for hp in range(HPAIRS):
                        qts[(sg, hp)] = qt_pool.tile(
                            [P, 512], MDT, tag="qt", name=f"qt{sg}_{hp}")
                xg = xgs[(t, sg)]
                psum = ps_main.tile([P, DG], F32, tag="ps")
                if use_bias:
                    nc.tensor.matmul(psum, ones_sb, bias_sb[ti:ti + 1, :],
                                     start=True, stop=False)
                for cc in range(CC):
                    nc.tensor.matmul(psum, xg[cc][:, sti * P:(sti + 1) * P],
                                     wg[(t, cc)],
                                     start=(cc == 0 and not use_bias),
                                     stop=(cc == CC - 1))
                if ti < 2:
                    flush_transposes()
                    nat = nat_pool.tile([P, DG], F32, tag="nat")
                    tmp = work_pool.tile([P, DG], F32, tag="w512")
                    tmp4 = tmp.rearrange("p (h t d) -> p h t d", h=HG, t=2)
                    ps4 = psum.rearrange("p (h t d) -> p h t d", h=HG, t=2)
                    nc.vector.tensor_mul(
                        nat.rearrange("p (h d) -> p h d", h=HG),
                        psum.rearrange("p (h d) -> p h d", h=HG),
                        cos_bc(st, None))
                    nc.vector.tensor_mul(tmp4[:, :, 0, :], ps4[:, :, 1, :],
                                         cos_bc(st, 0))
                    nc.vector.tensor_mul(tmp4[:, :, 1, :], ps4[:, :, 0, :],
                                         cos_bc(st, 1))
                    nc.vector.tensor_add(nat, nat, tmp)
                    pending_nat.append((ti, sg, sti, st, nat))
                else:
                    for h in range(HG):
                        nc.vector.tensor_copy(v4[:, h, st, 0:DK],
                                              psum[:, h * DK:(h + 1) * DK])

            def emit_outproj_unit(qr, sti, nr):
                st = qr * 4 + sti
                po = ps_main.tile([P, 512], F32, tag="ps")
                for dc in range(4):
                    nc.tensor.matmul(po, ctxs[(qr, dc)][:, sti * P:(sti + 1) * P],
                                     wo[(nr, dc)], start=(dc == 0),
                                     stop=(dc == 3))
                ot = work_pool.tile([P, 512], F32, tag="w512")
                nc.scalar.copy(ot, po)
                nc.sync.dma_start(
                    out=out_d[st * P:(st + 1) * P, nr * 512:(nr + 1) * 512],
                    in_=ot)

            is_ge = mybir.AluOpType.is_ge
            Exp = mybir.ActivationFunctionType.Exp
            Ln = mybir.ActivationFunctionType.Ln

            # prologue: projections for s-group 0, prefetch s-group 1
            for ti, t in enumerate("qkv"):
                emit_proj_dma(t, 0)
                for sti in range(4):
                    emit_proj_unit(ti, t, 0, sti)
            for t in "qkv":
                emit_proj_dma(t, 1)

            from functools import partial
            den_cur = []
            normalizers = deque()
            for qr in range(QR):
                flush_transposes()
                fillers = deque()
                if qr + 1 < QR:
                    for ti, t in enumerate("qkv"):
                        for sti in range(4):
                            fillers.append(
                                partial(emit_proj_unit, ti, t, qr + 1, sti))
                if qr + 2 < QR:
                    for t in "qkv":
                        fillers.append(partial(emit_proj_dma, t, qr + 2))
                if qr >= 1:
                    for sti in range(4):
                        for nr in range(2):
                            fillers.append(
                                partial(emit_outproj_unit, qr - 1, sti, nr))
                n_slots = 16 * (qr + 1)
                pace = max(1, n_slots // (len(fillers) + HPAIRS))
                cnt = 0
                for hp in range(HPAIRS):
                    hA, hB = 2 * hp, 2 * hp + 1
                    pcA = ps_ctx.tile([VSTRIDE, 512], F32, tag="pc")
                    pcB = ps_ctx.tile([VSTRIDE, 512], F32, tag="pc")
                    n_kc = 4 * (qr + 1)

                    def emit_attnv(eA, eB, kc, qoff, width, n_kc=n_kc,
                                   pcA=pcA, pcB=pcB, hA=hA, hB=hB):
                        nc.tensor.matmul(pcA[:, qoff:512],
                                         v4[:, hA, kc, :], eA,
                                         start=(kc == 0),
                                         stop=(kc == n_kc - 1),
                                         skip_group_check=True)
                        nc.tensor.matmul(pcB[:, qoff:512],
                                         v4[:, hB, kc, :], eB,
                                         start=(kc == 0),
                                         stop=(kc == n_kc - 1),
                                         skip_group_check=True)

                    pend = None
                    for kc in range(n_kc):
                        j = kc - 4 * qr
                        qoff = 128 * j if j > 0 else 0
                        width = 512 - qoff
                        psA = ps_main.tile([P, 512], F32, tag="ps")
                        psB = ps_main.tile([P, 512], F32, tag="ps")
                        ksl = slice(kc * P, (kc + 1) * P)
                        qt_r = qts[(qr, hp)]
                        nc.tensor.matmul(psA[:, qoff:512], kT[hp][0:64, ksl],
                                         qt_r[0:64, qoff:512],
                                         start=True, stop=True,
                                         tile_position=(0, 0))
                        nc.tensor.matmul(psB[:, qoff:512], kT[hp][64:128, ksl],
                                         qt_r[64:128, qoff:512],
                                         start=True, stop=True,
                                         tile_position=(64, 0))
                        eA = work_pool.tile([P, width], MDT, tag="w512",
                                            name="eA")
                        eB = work_pool.tile([P, width], MDT, tag="w512",
                                            name="eB")
                        nc.scalar.activation(out=eA, in_=psA[:, qoff:512],
                                             func=Exp, scale=0.125)
                        nc.scalar.activation(out=eB, in_=psB[:, qoff:512],
                                             func=Exp, scale=0.125)
                        if j >= 0:
                            for e in (eA, eB):
                                nc.gpsimd.affine_select(
                                    out=e, in_=e, compare_op=is_ge, fill=0.0,
                                    base=0, channel_multiplier=-1,
                                    pattern=[[1, width]])
                        if pend is not None:
                            emit_attnv(*pend)
                        pend = (eA, eB, kc, qoff, width)
                        cnt += 1
                        if cnt % pace == 0 and fillers:
                            fillers.popleft()()
                    emit_attnv(*pend)
                    denA = den_pool.tile([1, 512], F32, tag="rec", bufs=3,
                                         name="denA")
                    denB = den_pool.tile([1, 512], F32, tag="rec", bufs=3,
                                         name="denB")
                    nc.scalar.copy(denA, pcA[64:65, :])
                    nc.scalar.copy(denB, pcB[64:65, :])
                    ctx = ctx_pool.tile([P, 512], MDT, tag="ctx",
                                        name=f"ctx{qr}_{hp}")
                    ctxs[(qr, hp)] = ctx
                    nc.scalar.copy(ctx[0:64, :], pcA[0:64, :])
                    nc.scalar.copy(ctx[64:128, :], pcB[0:64, :])

                    def emit_normalize(ctx=ctx, denA=denA, denB=denB):
                        # broadcast raw dens to [128,512] via K=1 matmuls,
                        # then 1/x = exp(-ln x) on ACT: Ln/Exp share the
                        # natural_log_exp_and_others table set, so no table
                        # switches interleave with the softmax exps (the DVE
                        # InstReciprocal this replaces cost 3.4us per call).
                        pbc = ps_bc.tile([P, 512], F32, tag="pbc",
                                         name="pbc")
                        nc.tensor.matmul(pbc[0:64, :], ones1, denA,
                                         start=True, stop=True,
                                         tile_position=(0, 0),
                                         skip_group_check=True)
                        nc.tensor.matmul(pbc[64:128, :], ones1, denB,
                                         start=True, stop=True,
                                         tile_position=(0, 64),
                                         skip_group_check=True)
                        rbc = work_pool.tile([P, 512], F32, tag="rbc",
                                             bufs=2, name="rbc")
                        nc.scalar.activation(out=rbc, in_=pbc, func=Ln)
                        nc.scalar.activation(out=rbc, in_=rbc,
                                             func=Exp, scale=-1.0)
                        nc.gpsimd.tensor_mul(ctx[0:64, :], ctx[0:64, :],
                                             rbc[0:64, :])
                        nc.gpsimd.tensor_mul(ctx[64:128, :], ctx[64:128, :],
                                             rbc[64:128, :])

                    fillers.append(emit_normalize)
                while fillers:
                    fillers.popleft()()
            for sti in range(4):
                for nr in range(2):
                    emit_outproj_unit(QR - 1, sti, nr)
    return nc


def _build_program_v5(use_bias: bool, mm_dt: str = "fp32r"):
    """v4 plus: (a) Q/K projections emitted transposed (psum [dg, s])
    so the 128 PE transposes + 128 DVE evictions disappear; RoPE's
    rotate-half becomes one PE matmul against a constant +-1 permutation
    matrix plus 3 DVE ops against host-precomputed cosT/sinT [128, S]
    tables; (b) both heads' scores land in one 2-bank [128,1024] PSUM
    tile so each exp is a single paired ACTIVATE (saves the 352-cycle
    per-instruction ACT overhead) and each diagonal mask one
    affine_select; (c) e / V tiles in bf16 (frees ~25KB/partition SBUF,
    2x faster gpsimd selects; matmul rate unchanged); (d) ctx/den PSUM
    evictions moved from ACT to DVE."""
    from collections import deque
    from functools import partial

    import concourse.bass as bass
    import concourse.mybir as mybir
    import concourse.tile as tile

    F32 = mybir.dt.float32
    BF16 = mybir.dt.bfloat16
    MDT = mybir.dt.float32r if mm_dt == "fp32r" else F32

    def dcast(ap):
        return ap.bitcast(MDT) if MDT is not F32 else ap

    nc = bass.Bass()
    xs = {t: nc.dram_tensor(f"x{t}T", [D, S], F32, kind="ExternalInput")
          for t in "qkv"}
    ws = {t: nc.dram_tensor(f"w{t}T", [D, DG], F32, kind="ExternalInput")
          for t in "qkv"}
    woT = nc.dram_tensor("woT", [DG, D], F32, kind="ExternalInput")
    cosT_d = nc.dram_tensor("cosT_d", [P, S], F32, kind="ExternalInput")
    sinT_d = nc.dram_tensor("sinT_d", [P, S], F32, kind="ExternalInput")
    rotT_d = nc.dram_tensor("rotT_d", [P, P], F32, kind="ExternalInput")
    if use_bias:
        bias_d = nc.dram_tensor("bias_d", [4, DG], F32, kind="ExternalInput")
        ones_d = nc.dram_tensor("ones_d", [1, 512], F32, kind="ExternalInput")
    out_d = nc.dram_tensor("out", [S, D], F32, kind="ExternalOutput")

    with tile.TileContext(nc) as tc:
        with tc.tile_pool(name="consts", bufs=1) as consts, \
             tc.tile_pool(name="xT", bufs=8) as xT_pool, \
             tc.tile_pool(name="w", bufs=32) as w_pool, \
             tc.tile_pool(name="kt", bufs=4) as kt_pool, \
             tc.tile_pool(name="qt", bufs=8) as qt_pool, \
             tc.tile_pool(name="vp", bufs=1) as v_pool, \
             tc.tile_pool(name="ctx", bufs=8) as ctx_pool, \
             tc.tile_pool(name="den", bufs=1) as den_pool, \
             tc.tile_pool(name="wk", bufs=4) as work_pool, \
             tc.tile_pool(name="pss", bufs=2, space="PSUM") as ps_scores, \
             tc.tile_pool(name="psm", bufs=2, space="PSUM") as ps_small, \
             tc.tile_pool(name="psc", bufs=2, space="PSUM") as ps_ctx:

            ones1 = consts.tile([1, 64], F32)
            nc.vector.memset(ones1, 1.0)
            cosT = consts.tile([P, S], MDT)
            nc.sync.dma_start(out=cosT, in_=dcast(cosT_d[:, :]))
            sinT = consts.tile([P, S], MDT)
            nc.sync.dma_start(out=sinT, in_=dcast(sinT_d[:, :]))
            rotT = consts.tile([P, P], MDT)
            nc.sync.dma_start(out=rotT, in_=dcast(rotT_d[:, :]))
            if use_bias:
                bias_sb = consts.tile([4, DG], F32)
                nc.sync.dma_start(out=bias_sb, in_=bias_d[:, :])
                ones_sb = consts.tile([1, 512], F32)
                nc.sync.dma_start(out=ones_sb, in_=ones_d[:, :])

            kT = [kt_pool.tile([P, S], MDT, tag="kt", name=f"kT{i}")
                  for i in range(HPAIRS)]
            v_all = v_pool.tile([P, HG * ST * VSTRIDE], BF16)
            ones_col = consts.tile([P, 1], BF16)
            nc.vector.memset(ones_col, 1.0)
            ones_bc = bass.AP(tensor=ones_col.tensor, offset=ones_col.offset,
                              ap=[ones_col.ap[0], [0, HG], [0, ST], [0, 1]])
            v4 = v_all.rearrange("p (h t c) -> p h t c", h=HG, t=ST)
            nc.vector.tensor_copy(v4[:, :, :, DK:DK + 1], ones_bc)

            # all weights resident
            wg = {}
            for ti, t in enumerate("qkv"):
                for cc in range(CC):
                    wt = w_pool.tile([P, DG], MDT, tag="w", name=f"w{t}{cc}")
                    nc.sync.dma_start(out=wt,
                                      in_=dcast(ws[t][cc * P:(cc + 1) * P, :]))
                    wg[(t, cc)] = wt
            wo = {}
            for nr in range(2):
                for dc in range(4):
                    wt = w_pool.tile([P, 512], MDT, tag="w",
                                     name=f"wo{nr}_{dc}")
                    nc.sync.dma_start(
                        out=wt, in_=dcast(woT[dc * P:(dc + 1) * P,
                                               nr * 512:(nr + 1) * 512]))
                    wo[(nr, dc)] = wt

            qts = {}   # (sg, hp) -> [128, 512] MDT
            ctxs = {}  # (qr, hp) -> [128, 512] MDT
            xgs = {}   # (t, sg) -> chunk list

            def emit_proj_dma(t, sg):
                xg = [xT_pool.tile([P, 512], MDT, tag="xT",
                                   name=f"x{t}{sg}_{i}") for i in range(CC)]
                for cc in range(CC):
                    nc.sync.dma_start(
                        out=xg[cc],
                        in_=dcast(xs[t][cc * P:(cc + 1) * P,
                                        sg * 512:(sg + 1) * 512]))
                xgs[(t, sg)] = xg

            def emit_projT_unit(ti, t, sg, hp):
                # transposed Q/K projection: psT[dg(128 = head pair hp), s]
                if hp == 0 and ti == 0:
                    for h2 in range(HPAIRS):
                        qts[(sg, h2)] = qt_pool.tile(
                            [P, 512], MDT, tag="qt", name=f"qt{sg}_{h2}")
                xg = xgs[(t, sg)]
                psT = ps_small.tile([P, 512], F32, tag="ps", name="psT")
                if use_bias:
                    nc.tensor.matmul(psT, bias_sb[ti:ti + 1,
                                                  hp * P:(hp + 1) * P],
                                     ones_sb, start=True, stop=False,
                                     skip_group_check=True)
                for cc in range(CC):
                    nc.tensor.matmul(psT, wg[(t, cc)][:, hp * P:(hp + 1) * P],
                                     xg[cc],
                                     start=(cc == 0 and not use_bias),
                                     stop=(cc == CC - 1))
                # RoPE: dest = psT*cos + (rot @ psT)*sin
                raw = work_pool.tile([P, 512], MDT, tag="rope", bufs=2,
                                     name="raw")
                nc.vector.tensor_copy(raw, psT)
                rps = ps_small.tile([P, 512], F32, tag="ps", name="rps")
                nc.tensor.matmul(rps, rotT, raw, start=True, stop=True)
                dest = (qts[(sg, hp)] if ti == 0
                        else kT[hp][:, sg * 512:(sg + 1) * 512])
                ssl = slice(sg * 512, (sg + 1) * 512)
                nc.vector.tensor_mul(dest, raw, cosT[:, ssl])
                tmp = work_pool.tile([P, 512], MDT, tag="rope", bufs=2,
                                     name="rtmp")
                nc.vector.tensor_mul(tmp, rps.bitcast(MDT), sinT[:, ssl])
                nc.vector.tensor_add(dest, dest, tmp)

            def emit_projV_unit(t, sg, sti):
                st = sg * 4 + sti
                xg = xgs[(t, sg)]
                psum = ps_small.tile([P, DG], F32, tag="ps", name="psV")
                if use_bias:
                    nc.tensor.matmul(psum, ones_sb[0:1, 0:P],
                                     bias_sb[2:3, :], start=True, stop=False,
                                     skip_group_check=True)
                for cc in range(CC):
                    nc.tensor.matmul(psum, xg[cc][:, sti * P:(sti + 1) * P],
                                     wg[(t, cc)],
                                     start=(cc == 0 and not use_bias),
                                     stop=(cc == CC - 1))
                for h in range(HG):
                    nc.vector.tensor_copy(v4[:, h, st, 0:DK],
                                          psum[:, h * DK:(h + 1) * DK])

            def emit_outproj_unit(qr, sti, nr):
                st = qr * 4 + sti
                po = ps_small.tile([P, 512], F32, tag="ps", name="po")
                for dc in range(4):
                    nc.tensor.matmul(po, ctxs[(qr, dc)][:, sti * P:(sti + 1) * P],
                                     wo[(nr, dc)], start=(dc == 0),
                                     stop=(dc == 3))
                ot = work_pool.tile([P, 512], F32, tag="w512")
                nc.vector.tensor_copy(ot, po)
                nc.sync.dma_start(
                    out=out_d[st * P:(st + 1) * P, nr * 512:(nr + 1) * 512],
                    in_=ot)

            is_ge = mybir.AluOpType.is_ge
            Exp = mybir.ActivationFunctionType.Exp
            Ln = mybir.ActivationFunctionType.Ln

            # prologue: projections for s-group 0, prefetch s-group 1
            for ti, t in enumerate("qkv"):
                emit_proj_dma(t, 0)
                if ti < 2:
                    for hp in range(HPAIRS):
                        emit_projT_unit(ti, t, 0, hp)
                else:
                    for sti in range(4):
                        emit_projV_unit(t, 0, sti)
            for t in "qkv":
                emit_proj_dma(t, 1)

            for qr in range(QR):
                fillers = deque()
                if qr + 1 < QR:
                    for ti, t in enumerate("qkv"):
                        for u in range(4):
                            if ti < 2:
                                fillers.append(
                                    partial(emit_projT_unit, ti, t, qr + 1, u))
                            else:
                                fillers.append(
                                    partial(emit_projV_unit, t, qr + 1, u))
                if qr + 2 < QR:
                    for t in "qkv":
                        fillers.append(partial(emit_proj_dma, t, qr + 2))
                if qr >= 1:
                    for sti in range(4):
                        for nr in range(2):
                            fillers.append(
                                partial(emit_outproj_unit, qr - 1, sti, nr))
                n_slots = 16 * (qr + 1)
                pace = max(1, n_slots // (len(fillers) + HPAIRS))
                cnt = 0
                for hp in range(HPAIRS):
                    hA, hB = 2 * hp, 2 * hp + 1
                    pcA = ps_ctx.tile([VSTRIDE, 512], F32, tag="pc")
                    pcB = ps_ctx.tile([VSTRIDE, 512], F32, tag="pc")
                    n_kc = 4 * (qr + 1)

                    def emit_attnv(eAB, kc, qoff, width, n_kc=n_kc,
                                   pcA=pcA, pcB=pcB, hA=hA, hB=hB):
                        nc.tensor.matmul(pcA[:, qoff:512],
                                         v4[:, hA, kc, :], eAB[:, 0:width],
                                         start=(kc == 0),
                                         stop=(kc == n_kc - 1),
                                         skip_group_check=True)
                        nc.tensor.matmul(pcB[:, qoff:512],
                                         v4[:, hB, kc, :],
                                         eAB[:, width:2 * width],
                                         start=(kc == 0),
                                         stop=(kc == n_kc - 1),
                                         skip_group_check=True)

                    pend = None
                    for kc in range(n_kc):
                        j = kc - 4 * qr
                        qoff = 128 * j if j > 0 else 0
                        width = 512 - qoff
                        psAB = ps_scores.tile([P, 1024], F32, tag="pss")
                        ksl = slice(kc * P, (kc + 1) * P)
                        qt_r = qts[(qr, hp)]
                        nc.tensor.matmul(psAB[:, qoff:512],
                                         kT[hp][0:64, ksl],
                                         qt_r[0:64, qoff:512],
                                         start=True, stop=True,
                                         tile_position=(0, 0),
                                         skip_group_check=True)
                        nc.tensor.matmul(psAB[:, 512 + qoff:1024],
                                         kT[hp][64:128, ksl],
                                         qt_r[64:128, qoff:512],
                                         start=True, stop=True,
                                         tile_position=(64, 0),
                                         skip_group_check=True)
                        eAB = work_pool.tile([P, 2 * width], BF16,
                                             tag="eab", bufs=3, name="eAB")
                        ps3 = psAB.rearrange("p (b w) -> p b w", b=2)
                        e3 = eAB.rearrange("p (b w) -> p b w", b=2)
                        nc.scalar.activation(out=e3, in_=ps3[:, :, qoff:512],
                                             func=Exp, scale=0.125)
                        if j >= 0:
                            nc.gpsimd.affine_select(
                                out=e3, in_=e3, compare_op=is_ge, fill=0.0,
                                base=0, channel_multiplier=-1,
                                pattern=[[0, 2], [1, width]])
                        if pend is not None:
                            emit_attnv(*pend)
                        pend = (eAB, kc, qoff, width)
                        cnt += 1
                        if cnt % pace == 0 and fillers:
                            fillers.popleft()()
                    emit_attnv(*pend)
                    denA = den_pool.tile([1, 512], F32, tag="rec", bufs=3,
                                         name="denA")
                    denB = den_pool.tile([1, 512], F32, tag="rec", bufs=3,
                                         name="denB")
                    nc.vector.tensor_copy(denA, pcA[64:65, :])
                    nc.vector.tensor_copy(denB, pcB[64:65, :])
                    ctx = ctx_pool.tile([P, 512], MDT, tag="ctx",
                                        name=f"ctx{qr}_{hp}")
                    ctxs[(qr, hp)] = ctx
                    nc.vector.tensor_copy(ctx[0:64, :], pcA[0:64, :])
                    nc.vector.tensor_copy(ctx[64:128, :], pcB[0:64, :])

                    def emit_normalize(ctx=ctx, denA=denA, denB=denB):
                        # broadcast raw dens to [128,512] via K=1 matmuls,
                        # then 1/x = exp(-ln x) on ACT (Ln/Exp share the
                        # natural_log_exp_and_others table set: no switches)
                        pbc = ps_small.tile([P, 512], F32, tag="ps",
                                            name="pbc")
                        nc.tensor.matmul(pbc[0:64, :], ones1, denA,
                                         start=True, stop=True,
                                         tile_position=(0, 0),
                                         skip_group_check=True)
                        nc.tensor.matmul(pbc[64:128, :], ones1, denB,
                                         start=True, stop=True,
                                         tile_position=(0, 64),
                                         skip_group_check=True)
                        rbc = work_pool.tile([P, 512], F32, tag="rbc",
                                             bufs=2, name="rbc")
                        nc.scalar.activation(out=rbc, in_=pbc, func=Ln)
                        nc.scalar.activation(out=rbc, in_=rbc,
                                             func=Exp, scale=-1.0)
                        nc.gpsimd.tensor_mul(ctx[0:64, :], ctx[0:64, :],
                                             rbc[0:64, :])
                        nc.gpsimd.tensor_mul(ctx[64:128, :], ctx[64:128, :],
                                             rbc[64:128, :])

                    fillers.append(emit_normalize)
                while fillers:
                    fillers.popleft()()
            for sti in range(4):
                for nr in range(2):
                    emit_outproj_unit(QR - 1, sti, nr)
    return nc


_PROG_CACHE = {}


def _get_program(use_bias: bool):
    mm_dt = os.environ.get("KERNEL_MM_DT", "fp32r")
    ver = os.environ.get("KERNEL_V", "5")
    key = (use_bias, mm_dt, ver)
    if key not in _PROG_CACHE:
        if ver == "5":
            _PROG_CACHE[key] = _build_program_v5(use_bias, mm_dt=mm_dt)
        elif ver == "4":
            _PROG_CACHE[key] = _build_program_v4(use_bias, mm_dt=mm_dt)
        elif ver == "3":
            _PROG_CACHE[key] = _build_program_v3(use_bias, mm_dt=mm_dt)
        else:
            _PROG_CACHE[key] = _build_program(use_bias, mm_dt=mm_dt)
    return _PROG_CACHE[key]


def _rope_tables():
    inv = 1.0 / (ROPE_BASE ** (np.arange(0, DK, 2, dtype=np.float32) / DK))
    t = np.arange(S, dtype=np.float32)
    fr = t[:, None] * inv[None, :]                      # [S, 32]
    emb = np.concatenate([fr, fr], axis=-1)             # [S, 64]
    cos = np.cos(emb).astype(np.float32)
    sin = np.sin(emb).astype(np.float32)
    ssg = sin.copy()
    ssg[:, :32] = -sin[:, :32]
    return cos, ssg


def _rope_tables_T():
    """Transposed-layout RoPE tables for v5: cosT/sinT [128, S] (row r
    covers within-head dim r%64, duplicated across the 2 heads of a
    head-pair psum tile) and the +-1 rotate-half permutation rotT[j, i]
    (lhsT convention: out[i] = sum_j rotT[j, i] * in[j])."""
    inv = 1.0 / (ROPE_BASE ** (np.arange(0, DK, 2, dtype=np.float32) / DK))
    t = np.arange(S, dtype=np.float32)
    ang = t[None, :] * inv[np.arange(P) % 32][:, None]   # [128, S]
    cosT = np.cos(ang).astype(np.float32)
    sinT = np.sin(ang).astype(np.float32)
    rotT = np.zeros((P, P), np.float32)
    for b in (0, 64):
        for d in range(32):
            rotT[b + 32 + d, b + d] = -1.0
            rotT[b + d, b + 32 + d] = 1.0
    return cosT, sinT, rotT


def kernel(query, key, value, W_q, b_q, W_k, b_k, W_v, b_v, W_o, b_o):
    _install_patches()
    from concourse.bass_utils import run_bass_kernel_spmd

    query = np.asarray(query, dtype=np.float32)
    key = np.asarray(key, dtype=np.float32)
    value = np.asarray(value, dtype=np.float32)
    W_q, W_k, W_v, W_o = (np.asarray(w, dtype=np.float32)
                          for w in (W_q, W_k, W_v, W_o))
    b_q, b_k, b_v, b_o = (np.asarray(b, dtype=np.float32)
                          for b in (b_q, b_k, b_v, b_o))

    use_bias = bool(np.any(b_q) or np.any(b_k) or np.any(b_v))
    nc = _get_program(use_bias)
    ver = os.environ.get("KERNEL_V", "5")

    in_maps = []
    for c in range(N_CORES):
        b, g = divmod(c, 2)
        gs = slice(g * DG, (g + 1) * DG)
        m = {
            "xqT": np.ascontiguousarray(query[b].T),
            "xkT": np.ascontiguousarray(key[b].T),
            "xvT": np.ascontiguousarray(value[b].T),
            "wqT": np.ascontiguousarray(W_q[gs, :].T),
            "wkT": np.ascontiguousarray(W_k[gs, :].T),
            "wvT": np.ascontiguousarray(W_v[gs, :].T),
            "woT": np.ascontiguousarray(W_o[:, gs].T),
        }
        if ver == "5":
            cosT, sinT, rotT = _rope_tables_T()
            m["cosT_d"] = cosT
            m["sinT_d"] = sinT
            m["rotT_d"] = rotT
            if use_bias:
                m["bias_d"] = np.stack([b_q[gs], b_k[gs], b_v[gs],
                                        np.zeros(DG, np.float32)])
                m["ones_d"] = np.ones((1, 512), np.float32)
        else:
            cos, ssg = _rope_tables()
            m["cos_d"] = cos
            m["ssg_d"] = ssg
            if use_bias:
                m["bias_d"] = np.stack([b_q[gs], b_k[gs], b_v[gs],
                                        np.zeros(DG, np.float32)])
                m["ones_d"] = np.ones((1, P), np.float32)
        in_maps.append(m)

    trace = bool(int(os.environ.get("KERNEL_TRACE", "0")))
    trace_cores = None
    if trace:
        tc_env = os.environ.get("KERNEL_TRACE_CORES", "")
        trace_cores = ([int(x) for x in tc_env.split(",") if x != ""]
                       if tc_env else list(range(N_CORES)))
    try:
        res = run_bass_kernel_spmd(nc, in_maps, core_ids=list(range(N_CORES)),
                                   trace=trace, trace_cores=trace_cores)
    except Exception:
        if not trace:
            raise
        res = run_bass_kernel_spmd(nc, in_maps, core_ids=list(range(N_CORES)),
                                   trace=False)
    kernel._last_results = res

    out = np.empty((B, S, D), np.float32)
    for b in range(B):
        out[b] = res.results[2 * b]["out"] + res.results[2 * b + 1]["out"] + b_o
    return out

